# revision 1
# baseline (speedup 1.0000x reference)
"""GAT (3-layer, 4-head) + global mean pool + FC on 8 Trainium2 NeuronCores.

Strategy
--------
Nodes are sharded contiguously across 8 cores (2500 each; `batch` is sorted so
this is graph-aligned data parallelism per the hint, with cross-graph edges
handled exactly). Per layer:
  1. Each core computes H^T = W^T @ X^T for its node shard on the PE,
     plus per-node attention logits alpha_src/alpha_dst (tiny matmuls).
  2. H rows (bf16) and an alpha record table are AllGathered so every core
     holds the full 20000-row tables in local HBM.
  3. Edges are pre-sorted by destination on the host and grouped into
     128-destination windows x 128-edge chunks. Per chunk the core
     dma_gathers h[src] rows, computes q = exp(leakyrelu(as+ad)) densely,
     builds sparse alpha-scatter matrices S_h via tensor_scalar
     (iota == dstcol) * q, and PE matmuls S into PSUM: this performs the
     segment softmax numerator, denominator, and scatter-add in one pass.
  4. PSUM windows are normalized by the denominators, passed through ELU,
     and written back (already transposed) as the next layer's X^T.
Final: graph mean-pool via indicator matmuls, cross-core AllReduce, FC.
"""
import os
import sys
import time

sys.path.insert(0, "/opt/trn_rl_repo")

import ml_dtypes
import numpy as np

import concourse.bass as bass
import concourse.tile as tile
from concourse import bacc, mybir
from concourse.bass_utils import run_bass_kernel_spmd

# problem constants (hardcoded per the harness contract)
N = 20000
E0 = 320000
IN_CH = 256
HID = 128
HEADS = 4
OUT_CH = 200
G = 64
NEG_SLOPE = 0.2
NCORES = 8
SH = N // NCORES          # 2500 nodes per core
NW = (SH + 127) // 128    # 20 windows per core
SHP = NW * 128            # 2560 padded shard
P = 128

F32 = mybir.dt.float32
BF16 = mybir.dt.bfloat16
I16 = mybir.dt.int16
BF = ml_dtypes.bfloat16

AluOp = mybir.AluOpType
Act = mybir.ActivationFunctionType


# ----------------------------------------------------------------- host prep
def preprocess(edge_index, batch):
    src = np.concatenate([edge_index[0].astype(np.int64), np.arange(N)])
    dst = np.concatenate([edge_index[1].astype(np.int64), np.arange(N)])
    order = np.argsort(dst, kind="stable")
    src_s = src[order]
    dst_s = dst[order]

    core = dst_s // SH
    win = (dst_s % SH) // 128
    group = core * NW + win                      # 0..159, nondecreasing
    counts = np.bincount(group, minlength=NCORES * NW)
    K = int(np.ceil(counts.max() / 128))
    SLOTS = NW * K * 128

    starts = np.zeros(NCORES * NW, np.int64)
    starts[1:] = np.cumsum(counts)[:-1]
    rank = np.arange(len(dst_s)) - starts[group]
    slot = group * (K * 128) + rank              # global slot id

    SRC = np.zeros(NCORES * SLOTS, np.int64)
    DSTN = np.zeros(NCORES * SLOTS, np.int64)
    DCOL = np.full(NCORES * SLOTS, -1.0, np.float32)
    SRC[slot] = src_s
    DSTN[slot] = dst_s
    DCOL[slot] = (dst_s - core * SH - win * 128).astype(np.float32)

    def wrap16(a):
        # slot i -> [i%16, i//16], replicated to 128 partitions
        w = a.reshape(-1, 16).T.astype(np.int16)     # [16, SLOTS/16]
        return np.ascontiguousarray(np.tile(w, (8, 1)))

    per_core = []
    for c in range(NCORES):
        sl = slice(c * SLOTS, (c + 1) * SLOTS)
        srcidx = wrap16(SRC[sl])                      # [128, SLOTS/16] i16
        dstidx = wrap16(DSTN[sl])
        # dstcol [128, NW*K]: slot (w,k,p) -> [p, w*K+k]
        dcol = (
            DCOL[sl].reshape(NW, K, 128).transpose(2, 0, 1).reshape(128, NW * K)
        )
        nodes = c * SH + np.arange(SHP)
        gid = np.where(nodes < (c + 1) * SH, batch[np.minimum(nodes, N - 1)], -1)
        gidcol = (
            gid.reshape(NW, 128).T.astype(np.float32)
        )  # [128, NW]
        per_core.append(
            dict(
                srcidx=srcidx,
                dstidx=dstidx,
                dstcol=np.ascontiguousarray(dcol),
                gidcol=np.ascontiguousarray(gidcol),
            )
        )
    cnts = np.bincount(batch.astype(np.int64), minlength=G).astype(np.float32)
    invcnt = (1.0 / np.maximum(cnts, 1.0)).reshape(G, 1)
    return K, per_core, invcnt


# ------------------------------------------------------------ device program
def build_program(K, stages=99):
    nc = bacc.Bacc("TRN2", num_devices=NCORES)
    IDXW = NW * K * 128 // 16   # idx cols per core

    # ---- inputs
    xT = nc.dram_tensor("xT", [IN_CH, SHP], BF16, kind="ExternalInput")
    w1 = nc.dram_tensor("w1", [P, 2, 512], BF16, kind="ExternalInput")
    w2 = nc.dram_tensor("w2", [P, 4, 512], BF16, kind="ExternalInput")
    w3 = nc.dram_tensor("w3", [P, 4, 128], BF16, kind="ExternalInput")
    a1 = nc.dram_tensor("a1", [P, 8], BF16, kind="ExternalInput")
    a2 = nc.dram_tensor("a2", [P, 8], BF16, kind="ExternalInput")
    a3 = nc.dram_tensor("a3", [P, 2], BF16, kind="ExternalInput")
    srcidx_d = nc.dram_tensor("srcidx", [P, IDXW], I16, kind="ExternalInput")
    dstidx_d = nc.dram_tensor("dstidx", [P, IDXW], I16, kind="ExternalInput")
    dstcol_d = nc.dram_tensor("dstcol", [P, NW * K], F32, kind="ExternalInput")
    gidcol_d = nc.dram_tensor("gidcol", [P, NW], F32, kind="ExternalInput")
    iota128_d = nc.dram_tensor("iota128", [P, P], BF16, kind="ExternalInput")
    iota64_d = nc.dram_tensor("iota64", [P, G], BF16, kind="ExternalInput")
    ones_d = nc.dram_tensor("onesrow", [1, P], BF16, kind="ExternalInput")
    idbf_d = nc.dram_tensor("idbf", [P, P], BF16, kind="ExternalInput")
    idf32_d = nc.dram_tensor("idf32", [P, P], F32, kind="ExternalInput")
    invcnt_d = nc.dram_tensor("invcnt", [G, 1], F32, kind="ExternalInput")
    fcw_d = nc.dram_tensor("fcw", [HID, OUT_CH], F32, kind="ExternalInput")
    out_d = nc.dram_tensor("logits", [G, OUT_CH], F32, kind="ExternalOutput")
    xdump_d = nc.dram_tensor("xdump", [P, 4, SHP], BF16, kind="ExternalOutput")
    hdump_d = nc.dram_tensor("hdump", [N, 512], BF16, kind="ExternalOutput")
    gdump_d = nc.dram_tensor("gdump", [P, K, 512 + 256], BF16, kind="ExternalOutput")
    qdump_d = nc.dram_tensor("qdump", [P, K, 8], F32, kind="ExternalOutput")
    adump_d = nc.dram_tensor("adump", [N, 128], BF16, kind="ExternalOutput")

    # ---- DRAM internals / collective buffers
    hag_in = [
        nc.dram_tensor(f"hag_in{l}", [SH, 512 if l < 3 else 128], BF16)
        for l in (1, 2, 3)
    ]
    hag_out = [
        nc.dram_tensor(
            f"hag_out{l}", [N, 512 if l < 3 else 128], BF16, addr_space="Shared"
        )
        for l in (1, 2, 3)
    ]
    aag_in = [nc.dram_tensor(f"aag_in{l}", [SH, 128], BF16) for l in (1, 2, 3)]
    aag_out = [
        nc.dram_tensor(f"aag_out{l}", [N, 128], BF16, addr_space="Shared")
        for l in (1, 2, 3)
    ]
    ar_in = nc.dram_tensor("ar_in", [G, HID], F32)
    ar_out = nc.dram_tensor("ar_out", [G, HID], F32, addr_space="Shared")

    RG = [list(range(NCORES))]

    with tile.TileContext(nc) as tc:
        with (
            tc.tile_pool(name="const", bufs=1) as cpool,
            tc.tile_pool(name="xbuf", bufs=1) as xpool,
            tc.tile_pool(name="mm", bufs=2) as mmpool,
            tc.tile_pool(name="gather", bufs=2) as gpool,
            tc.tile_pool(name="chunk", bufs=3) as kpool,
            tc.tile_pool(name="post", bufs=2) as ppool,
            tc.tile_pool(name="psA", bufs=1, space="PSUM") as psA,
        ):
            # ---- constants into SBUF
            def load_const(dram, shape, dt, name):
                t = cpool.tile(shape, dt, name=name)
                nc.sync.dma_start(t[:], dram[:])
                return t

            iota128 = load_const(iota128_d, [P, P], BF16, "iota128")
            iota64 = load_const(iota64_d, [P, G], BF16, "iota64")
            onesrow = load_const(ones_d, [1, P], BF16, "onesrow")
            onescol = cpool.tile([P, 1], BF16, name="onescol")
            nc.vector.memset(onescol[:], 1.0)
            idbf = load_const(idbf_d, [P, P], BF16, "idbf")
            idf32 = load_const(idf32_d, [P, P], F32, "idf32")
            invcnt = load_const(invcnt_d, [G, 1], F32, "invcnt")
            fcw = load_const(fcw_d, [HID, OUT_CH], F32, "fcw")
            a_sb = [
                load_const(d, [P, n], BF16, f"a{i+1}")
                for i, (d, n) in enumerate([(a1, 8), (a2, 8), (a3, 2)])
            ]
            w_sb = [
                load_const(w1, [P, 2, 512], BF16, "w1"),
                load_const(w2, [P, 4, 512], BF16, "w2"),
                load_const(w3, [P, 4, 128], BF16, "w3"),
            ]
            srcidx = load_const(srcidx_d, [P, IDXW], I16, "srcidx")
            dstidx = load_const(dstidx_d, [P, IDXW], I16, "dstidx")
            dstcol = load_const(dstcol_d, [P, NW * K], F32, "dstcol")
            gidcol = load_const(gidcol_d, [P, NW], F32, "gidcol")

            # ---- X^T buffers
            x1 = xpool.tile([P, 2, SHP], BF16, tag="xA", name="x1")
            nc.sync.dma_start(x1[:], xT[:].rearrange("(c p) n -> p c n", p=128))
            x2 = xpool.tile([P, 4, SHP], BF16, tag="xB", name="x2")
            x3 = xpool.tile([P, 4, SHP], BF16, tag="xA", name="x3")
            x4 = xpool.tile([P, 1, SHP], BF16, tag="xD", name="x4")
            xbufs = [x1, x2, x3, x4]

            NT = SHP // 512  # 5 moving tiles per shard

            def matmul_phase(l):
                """X^T -> H^T (sbuf), alpha records -> aag_in, H rows -> hag_in."""
                cinb = [2, 4, 4][l]
                coutb = [4, 4, 1][l]
                nh = [4, 4, 1][l]
                xin = xbufs[l]
                ht = mmpool.tile([P, coutb, SHP], BF16, tag="ht", bufs=1, name=f"ht{l}")
                for co in range(coutb):
                    for t in range(NT):
                        pm = psA.tile([P, 512], F32, tag=f"agg{(co * NT + t) % 2}",
                                      name=f"mmps{l}_{co}_{t}")
                        for k in range(cinb):
                            nc.tensor.matmul(
                                out=pm[:],
                                lhsT=w_sb[l][:, k, co * 128 : co * 128 + 128],
                                rhs=xin[:, k, t * 512 : (t + 1) * 512],
                                start=(k == 0),
                                stop=(k == cinb - 1),
                            )
                        nc.scalar.activation(
                            ht[:, co, t * 512 : (t + 1) * 512], pm[:], Act.Copy
                        )
                # alpha logits: aT[r, n] for r in (src heads | dst heads)
                for t in range(NT):
                    ast_h = []
                    for h in range(nh):
                        pa = psA.tile([2, 512], F32, tag="den", name=f"aps{l}_{t}_{h}")
                        nc.tensor.matmul(
                            out=pa[:],
                            lhsT=a_sb[l][:, 2 * h : 2 * h + 2],
                            rhs=ht[:, h, t * 512 : (t + 1) * 512],
                            start=True,
                            stop=True,
                        )
                        ah = ppool.tile([2, 512], F32, tag=f"astage{h}", bufs=1, name=f"ast{l}_{t}_{h}")
                        nc.vector.tensor_copy(ah[:], pa[:])
                        ast_h.append(ah)
                    # transpose to per-node records and store as bf16-bitcast
                    for b in range(4):  # 4 x 128 nodes per 512-tile
                        arec = ppool.tile([P, 2 * nh], F32, tag="arec", name=f"are{l}_{t}_{b}")
                        for h in range(nh):
                            pt = psA.tile([P, 2], F32, tag="dbc", name=f"atp{l}_{t}_{b}_{h}")
                            nc.tensor.matmul(
                                out=pt[:],
                                lhsT=ast_h[h][:, b * 128 : (b + 1) * 128],
                                rhs=idf32[:2, :2],
                                start=True,
                                stop=True,
                                is_transpose=True,
                            )
                            nc.vector.tensor_copy(arec[:, 2 * h : 2 * h + 2], pt[:])
                        w_ = t * 4 + b
                        lo = w_ * 128
                        hi = min(lo + 128, SH)
                        if hi <= lo:
                            continue
                        nc.sync.dma_start(
                            aag_in[l][lo:hi, : 4 * nh].bitcast(F32),
                            arec[: hi - lo, :],
                        )
                # H rows: transpose H^T window blocks and write hag_in
                for w in range(NW):
                    hst = ppool.tile([P, coutb * 128], BF16, tag="hstage", name=f"hs{l}_{w}")
                    for co in range(coutb):
                        pt = psA.tile([P, P], BF16, tag=f"agg{2 + co % 2}", name=f"htp{l}_{w}_{co}")
                        nc.tensor.matmul(
                            out=pt[:],
                            lhsT=ht[:, co, w * 128 : (w + 1) * 128],
                            rhs=idbf[:],
                            start=True,
                            stop=True,
                            is_transpose=True,
                        )
                        nc.vector.tensor_copy(
                            hst[:, co * 128 : (co + 1) * 128], pt[:]
                        )
                    lo = w * 128
                    hi = min(lo + 128, SH)
                    nc.sync.dma_start(hag_in[l][lo:hi, :], hst[: hi - lo, :])
                nc.gpsimd.collective_compute(
                    "AllGather", AluOp.bypass, replica_groups=RG,
                    ins=[hag_in[l][:]], outs=[hag_out[l][:]],
                )
                nc.gpsimd.collective_compute(
                    "AllGather", AluOp.bypass, replica_groups=RG,
                    ins=[aag_in[l][:]], outs=[aag_out[l][:]],
                )

            def agg_phase(l):
                """Gather + attention + scatter-add; writes next X^T (elu'd)."""
                nh = [4, 4, 1][l]
                C = [512, 512, 128][l]
                xout = xbufs[l + 1]
                NI = K * 128
                agg_nw = int(os.environ.get("AGG_NW", NW))
                agg_parts = int(os.environ.get("AGG_PARTS", 7))
                for w in range(NW):
                    if w >= agg_nw:
                        nc.vector.memset(xout[:, :, w * 128 : (w + 1) * 128], 0.0)
                        continue
                    isl = slice(w * (NI // 16), (w + 1) * (NI // 16))
                    ngath = int(os.environ.get("AGG_G", 3))
                    six = gpool.tile([P, NI // 16], I16, tag="six", bufs=2, name=f"six{l}_{w}")
                    nc.vector.tensor_copy(six[:], srcidx[:, isl])
                    dix = gpool.tile([P, NI // 16], I16, tag="dix", bufs=2, name=f"dix{l}_{w}")
                    nc.vector.tensor_copy(dix[:], dstidx[:, isl])
                    hg = gpool.tile([P, K, C], BF16, tag="hg", name=f"hg{l}_{w}")
                    if ngath >= 1:
                        nc.gpsimd.dma_gather(
                            hg[:], hag_out[l][:], six[:], NI, NI, C,
                            single_packet=False,
                        )
                    asg = gpool.tile([P, K, 128], BF16, tag="asg", bufs=1, name=f"as{l}_{w}")
                    if ngath >= 2:
                        nc.gpsimd.dma_gather(
                            asg[:], aag_out[l][:], six[:], NI, NI, 128,
                            single_packet=False,
                        )
                    adg = gpool.tile([P, K, 128], BF16, tag="adg", bufs=1, name=f"ad{l}_{w}")
                    if ngath >= 3:
                        nc.gpsimd.dma_gather(
                            adg[:], aag_out[l][:], dix[:], NI, NI, 128,
                            single_packet=False,
                        )
                    if not (agg_parts & 2):
                        nc.vector.memset(xout[:, :, w * 128 : (w + 1) * 128], 0.0)
                        continue
                    # q = exp(lrelu(as+ad)) [128, K, nh]
                    asf = asg[:].bitcast(F32).rearrange(
                        "p k (c two) -> p k c two", two=2
                    )  # [P, K, 32, 2]
                    adf = adg[:].bitcast(F32).rearrange(
                        "p k (c two) -> p k c two", two=2
                    )
                    qt = ppool.tile([P, K, nh], F32, tag="qt", name=f"qt{l}_{w}")
                    nc.vector.tensor_tensor(
                        out=qt[:], in0=asf[:, :, 0:nh, 0], in1=adf[:, :, 0:nh, 1],
                        op=AluOp.add,
                    )
                    qsc = ppool.tile([P, K, nh], F32, tag="qsc", bufs=1, name=f"qs{l}_{w}")
                    nc.vector.tensor_scalar(
                        out=qsc[:], in0=qt[:], scalar1=NEG_SLOPE, scalar2=None,
                        op0=AluOp.mult,
                    )
                    qlr = ppool.tile([P, K, nh], F32, tag="qlr", name=f"ql{l}_{w}")
                    nc.vector.tensor_tensor(
                        out=qlr[:], in0=qt[:], in1=qsc[:], op=AluOp.max,
                    )
                    qf = ppool.tile([P, K, nh], F32, tag="qf", name=f"qf{l}_{w}")
                    nc.scalar.activation(qf[:], qlr[:], Act.Exp)

                    if l == 0 and w == int(os.environ.get("DUMP_W", -1)):
                        nc.sync.dma_start(gdump_d[:, :, 0:C], hg[:])
                        nc.sync.dma_start(gdump_d[:, :, 512:640], asg[:])
                        nc.sync.dma_start(gdump_d[:, :, 640:768], adg[:])
                        nc.sync.dma_start(qdump_d[:, :, 0:nh], qf[:])
                    pagg = [
                        psA.tile([P, P], F32, tag=f"agg{h}", name=f"pa{l}_{w}_{h}")
                        for h in range(nh)
                    ]
                    pden = psA.tile([1, nh * P], F32, tag="den", name=f"pd{l}_{w}")
                    for k in range(K):
                        S4 = kpool.tile([P, nh * P], BF16, tag="S4", name=f"S{l}_{w}_{k}")
                        for h in range(nh):
                            nc.vector.tensor_scalar(
                                out=S4[:, h * P : (h + 1) * P],
                                in0=iota128[:],
                                scalar1=dstcol[:, w * K + k : w * K + k + 1],
                                scalar2=qf[:, k, h : h + 1],
                                op0=AluOp.is_equal,
                                op1=AluOp.mult,
                            )
                        for h in range(nh):
                            nc.tensor.matmul(
                                out=pagg[h][:],
                                lhsT=hg[:, k, h * P : (h + 1) * P],
                                rhs=S4[:, h * P : (h + 1) * P],
                                start=(k == 0),
                                stop=(k == K - 1),
                            )
                        nc.tensor.matmul(
                            out=pden[:],
                            lhsT=onescol[:],
                            rhs=S4[:],
                            start=(k == 0),
                            stop=(k == K - 1),
                        )
                    if not (agg_parts & 4):
                        nc.vector.memset(xout[:, :, w * 128 : (w + 1) * 128], 0.0)
                        # still must close psum accumulation groups: read them
                        sink = ppool.tile([P, nh * P], F32, tag="tt", bufs=1, name=f"sink{l}_{w}")
                        for h in range(nh):
                            nc.vector.tensor_copy(sink[:, h * P : (h + 1) * P], pagg[h][:])
                        sden = ppool.tile([1, nh * P], F32, tag="densb", bufs=1, name=f"sd{l}_{w}")
                        nc.vector.tensor_copy(sden[:], pden[:])
                        continue
                    # normalize + elu -> xout[:, :, w*128...]
                    den = ppool.tile([1, nh * P], F32, tag="densb", bufs=1, name=f"dn{l}_{w}")
                    nc.vector.tensor_scalar(
                        out=den[:], in0=pden[:], scalar1=1e-16, scalar2=None,
                        op0=AluOp.add,
                    )
                    rec = ppool.tile([1, nh * P], F32, tag="rec", bufs=1, name=f"rc{l}_{w}")
                    nc.vector.reciprocal(rec[:], den[:])
                    recbf = ppool.tile([1, nh * P], BF16, tag="recbf", bufs=1, name=f"rb{l}_{w}")
                    nc.vector.tensor_copy(recbf[:], rec[:])
                    pdbc = psA.tile([P, nh * P], F32, tag="dbc", name=f"db{l}_{w}")
                    for h in range(nh):
                        nc.tensor.matmul(
                            out=pdbc[:, h * P : (h + 1) * P],
                            lhsT=onesrow[:],
                            rhs=recbf[:, h * P : (h + 1) * P],
                            start=True,
                            stop=True,
                        )
                    dbc = ppool.tile([P, nh * P], F32, tag="dbcsb", bufs=1, name=f"dc{l}_{w}")
                    nc.scalar.activation(dbc[:], pdbc[:], Act.Copy)
                    tt = ppool.tile([P, nh * P], F32, tag="tt", bufs=1, name=f"tt{l}_{w}")
                    for h in range(nh):
                        nc.vector.tensor_tensor(
                            out=tt[:, h * P : (h + 1) * P],
                            in0=pagg[h][:],
                            in1=dbc[:, h * P : (h + 1) * P],
                            op=AluOp.mult,
                        )
                    # elu(t) = exp(min(t,0)) - 1 + max(t,0) = em + max(t-1,-1)
                    mm_ = ppool.tile([P, nh * P], F32, tag="elm", bufs=1, name=f"em{l}_{w}")
                    nc.vector.tensor_scalar(
                        out=mm_[:], in0=tt[:], scalar1=0.0, scalar2=None,
                        op0=AluOp.min,
                    )
                    em = ppool.tile([P, nh * P], F32, tag="elexp", bufs=1, name=f"ee{l}_{w}")
                    nc.scalar.activation(em[:], mm_[:], Act.Exp)
                    r1 = ppool.tile([P, nh * P], F32, tag="elr", bufs=1, name=f"er{l}_{w}")
                    nc.vector.tensor_scalar(
                        out=r1[:], in0=tt[:], scalar1=-1.0, scalar2=-1.0,
                        op0=AluOp.add, op1=AluOp.max,
                    )
                    nc.vector.tensor_tensor(
                        out=xout[:, :, w * 128 : (w + 1) * 128],
                        in0=em[:],
                        in1=r1[:],
                        op=AluOp.add,
                    )

            stage = 0
            done = False
            for l in range(3):
                if stage >= stages:
                    done = True
                    break
                matmul_phase(l)
                stage += 1
                if stage >= stages:
                    done = True
                    break
                agg_phase(l)
                stage += 1

            # ---- pool + FC
            def pool_fc():
                ppsum = psA.tile([G, HID], F32, tag="agg0", name="poolps")
                for w in range(NW):
                    pt = psA.tile([P, P], BF16, tag=f"agg{2 + w % 2}", name=f"xt4_{w}")
                    nc.tensor.matmul(
                        out=pt[:], lhsT=x4[:, 0, w * 128 : (w + 1) * 128], rhs=idbf[:],
                        start=True, stop=True, is_transpose=True,
                    )
                    x4w = ppool.tile([P, P], BF16, tag="x4w", name=f"x4w_{w}")
                    nc.vector.tensor_copy(x4w[:], pt[:])
                    gw = ppool.tile([P, G], BF16, tag="gw", name=f"gw_{w}")
                    nc.vector.tensor_scalar(
                        out=gw[:], in0=iota64[:], scalar1=gidcol[:, w : w + 1],
                        scalar2=None, op0=AluOp.is_equal,
                    )
                    nc.tensor.matmul(
                        out=ppsum[:], lhsT=gw[:], rhs=x4w[:],
                        start=(w == 0), stop=(w == NW - 1),
                    )
                psums = ppool.tile([G, HID], F32, tag="psums", name="psums")
                nc.vector.tensor_copy(psums[:], ppsum[:])
                nc.sync.dma_start(ar_in[:], psums[:])
                nc.gpsimd.collective_compute(
                    "AllReduce", AluOp.add, replica_groups=RG,
                    ins=[ar_in[:]], outs=[ar_out[:]],
                )
                sums = ppool.tile([G, HID], F32, tag="sums", name="sums")
                nc.sync.dma_start(sums[:], ar_out[:])
                pooled = ppool.tile([G, HID], F32, tag="pooled", name="pooled")
                nc.vector.tensor_scalar(
                    out=pooled[:], in0=sums[:], scalar1=invcnt[:, 0:1], scalar2=None,
                    op0=AluOp.mult,
                )
                ptp = psA.tile([HID, G], F32, tag="den", name="poolT")
                nc.tensor.matmul(
                    out=ptp[:], lhsT=pooled[:], rhs=idf32[:G, :G],
                    start=True, stop=True, is_transpose=True,
                )
                poolT = ppool.tile([HID, G], F32, tag="poolT", name="poolTs")
                nc.vector.tensor_copy(poolT[:], ptp[:])
                pfc = psA.tile([G, OUT_CH], F32, tag="dbc", name="fcps")
                nc.tensor.matmul(
                    out=pfc[:], lhsT=poolT[:], rhs=fcw[:], start=True, stop=True
                )
                logits = ppool.tile([G, OUT_CH], F32, tag="logits", name="logits")
                nc.vector.tensor_copy(logits[:], pfc[:])
                nc.sync.dma_start(out_d[:], logits[:])

            def debug_dump():
                dbg = ppool.tile([1, OUT_CH], F32, tag="dbg", bufs=1, name="dbg")
                nc.vector.memset(dbg[:], 7.0)
                nc.sync.dma_start(out_d[0:1, :], dbg[:])

            if os.environ.get("DUMP_H"):
                li = int(os.environ["DUMP_H"]) - 1
                cw = 512 if li < 2 else 128
                hstg = ppool.tile([P, cw], BF16, tag="hdmp", bufs=2, name="hdmp")
                astg = ppool.tile([P, 128], BF16, tag="admp", bufs=2, name="admp")
                for b in range((N + P - 1) // P):
                    lo2, hi2 = b * P, min((b + 1) * P, N)
                    nn = hi2 - lo2
                    nc.sync.dma_start(hstg[:nn, :], hag_out[li][lo2:hi2, :])
                    nc.sync.dma_start(hdump_d[lo2:hi2, 0:cw], hstg[:nn, :])
                    nc.sync.dma_start(astg[:nn, :], aag_out[li][lo2:hi2, :])
                    nc.sync.dma_start(adump_d[lo2:hi2, :], astg[:nn, :])
            if os.environ.get("DUMP_X"):
                xi = int(os.environ["DUMP_X"])
                src = xbufs[xi]
                cb = src.shape[1]
                nc.sync.dma_start(xdump_d[:, 0:cb, :], src[:])
            if done:
                debug_dump()
            else:
                pool_fc()

    nc.compile()
    return nc


_prog_cache = {}


def kernel(x, edge_index, batch, W1, a_src1, a_dst1, b1,
           W2, a_src2, a_dst2, b2, W3, a_src3, a_dst3, b3, fc_w, fc_b,
           _want_results=False, _trace=False, _stages=99):
    x = np.asarray(x)
    edge_index = np.asarray(edge_index)
    batch = np.asarray(batch)
    for b in (b1, b2, b3, fc_b):
        assert not np.any(np.asarray(b)), "nonzero biases not supported"

    K, per_core, invcnt = preprocess(edge_index, batch)
    ck = (K, _stages)
    if ck not in _prog_cache:
        _prog_cache[ck] = build_program(K, _stages)
    nc = _prog_cache[ck]

    iota128 = np.ascontiguousarray(
        np.broadcast_to(np.arange(P, dtype=np.float32), (P, P)).astype(BF)
    )
    iota64 = np.ascontiguousarray(
        np.broadcast_to(np.arange(G, dtype=np.float32), (P, G)).astype(BF)
    )
    onesrow = np.ones((1, P), BF)
    idbf = np.eye(P, dtype=np.float32).astype(BF)
    idf32 = np.eye(P, dtype=np.float32)

    def wmat(W, cinb, cout):
        return np.ascontiguousarray(
            np.asarray(W, np.float32).reshape(cinb, 128, cout).transpose(1, 0, 2)
        ).astype(BF)

    w1m = wmat(W1, 2, 512)
    w2m = wmat(W2, 4, 512)
    w3m = wmat(W3, 4, 128)

    def avec(asrc, adst):
        # [128, 2*nh]: cols interleaved (src_h, dst_h) pairs; rows = channel
        nh = asrc.shape[0]
        out = np.empty((128, 2 * nh), np.float32)
        out[:, 0::2] = np.asarray(asrc, np.float32).T
        out[:, 1::2] = np.asarray(adst, np.float32).T
        return np.ascontiguousarray(out).astype(BF)

    a1m = avec(a_src1, a_dst1)
    a2m = avec(a_src2, a_dst2)
    a3m = avec(a_src3, a_dst3)
    fcw = np.ascontiguousarray(np.asarray(fc_w, np.float32))

    xf = np.asarray(x, np.float32)
    in_maps = []
    for c in range(NCORES):
        xs = np.zeros((IN_CH, SHP), np.float32)
        xs[:, :SH] = xf[c * SH : (c + 1) * SH].T
        pc = per_core[c]
        in_maps.append(
            dict(
                xT=xs.astype(BF),
                w1=w1m, w2=w2m, w3=w3m, a1=a1m, a2=a2m, a3=a3m,
                srcidx=pc["srcidx"], dstidx=pc["dstidx"],
                dstcol=pc["dstcol"], gidcol=pc["gidcol"],
                iota128=iota128, iota64=iota64, onesrow=onesrow,
                idbf=idbf, idf32=idf32, invcnt=invcnt, fcw=fcw,
            )
        )
    res = run_bass_kernel_spmd(
        nc, in_maps, list(range(NCORES)), trace=_trace
    )
    out = res.results[0]["logits"].astype(np.float32)
    if _want_results:
        return out, res
    return out



# revision 10
# speedup vs baseline: 2.0681x; 2.0681x over previous
"""GAT (3-layer, 4-head) + global mean pool + FC on 8 Trainium2 NeuronCores.

Strategy (v2)
-------------
Nodes sharded contiguously across 8 cores (2500 each, padded to 2560; `batch`
is sorted so this is graph-aligned data parallelism). Per layer:
  1. Dense phase: H^T = W^T X^T on the PE; attention logits a_src/a_dst per
     node; H^T transposed into a per-node row table [h(interleaved)|a_src]
     and AllGathered so every core holds the full 20480-row table in HBM.
  2. Agg phase, per 128-dst window: ONE dma_gather fetches h+a_src rows for
     the window's edges (sorted by dst, chunked 128/chunk). Host-precomputed
     one-hot scatter matrices O / O^T (static edge structure) stream in via
     HWDGE. a_dst per edge = O^T @ a_dst_window on the PE; q = exp(lrelu(
     a_s+a_d)) on ACT; q folded into gathered rows with a single broadcast
     multiply per chunk (head-interleaved channels); PE contracts
     out[dst,c] += O^T(edges->dst) . (q*h) plus denominators. Normalize +
     ELU fused on full-width [128,512] ops with per-partition reciprocals.
  3. Next layer's X^T obtained by HWDGE dma-transpose of the row output.
Final: graph mean-pool fused into layer-3 agg windows, AllReduce, FC.
"""
import os
import sys

sys.path.insert(0, "/opt/trn_rl_repo")

import ml_dtypes
import numpy as np

import concourse.bass as bass
import concourse.tile as tile
from concourse import bacc, mybir
from concourse.bass_utils import run_bass_kernel_spmd

# problem constants (hardcoded per the harness contract)
N = 20000
E0 = 320000
IN_CH = 256
HID = 128
HEADS = 4
OUT_CH = 200
G = 64
NEG_SLOPE = 0.2
NCORES = 8
SH = N // NCORES          # 2500 nodes per core
NW = (SH + 127) // 128    # 20 windows per core
SHP = NW * 128            # 2560 padded shard
NP = NCORES * SHP         # 20480 padded global rows
P = 128
ROW1 = 640                # table row cols (bf16) for layers 1-2: 512 h + 8 as-f32 + pad
ROW3 = 256                # layer 3: 128 h + 2 as-f32 + pad

F32 = mybir.dt.float32
BF16 = mybir.dt.bfloat16
I16 = mybir.dt.int16
BF = ml_dtypes.bfloat16

AluOp = mybir.AluOpType
Act = mybir.ActivationFunctionType


# ----------------------------------------------------------------- host prep
def preprocess(edge_index, batch):
    src = np.concatenate([edge_index[0].astype(np.int64), np.arange(N)])
    dst = np.concatenate([edge_index[1].astype(np.int64), np.arange(N)])
    order = np.argsort(dst, kind="stable")
    src_s = src[order]
    dst_s = dst[order]

    core = dst_s // SH
    win = (dst_s % SH) // 128
    group = core * NW + win                      # 0..159, nondecreasing
    counts = np.bincount(group, minlength=NCORES * NW)
    K = int(np.ceil(counts.max() / 128))
    SLOTS = NW * K * 128

    starts = np.zeros(NCORES * NW, np.int64)
    starts[1:] = np.cumsum(counts)[:-1]
    rank = np.arange(len(dst_s)) - starts[group]
    slot = group * (K * 128) + rank              # global slot id

    SRC = np.zeros(NCORES * SLOTS, np.int64)
    DCOL = np.full(NCORES * SLOTS, -1, np.int64)
    # remap src node id into the padded 2560-per-core row space
    SRC[slot] = (src_s // SH) * SHP + (src_s % SH)
    DCOL[slot] = dst_s - core * SH - win * 128

    def wrap16(a):
        # slot i -> [i%16, i//16], replicated to 128 partitions
        w = a.reshape(-1, 16).T.astype(np.int16)     # [16, SLOTS/16]
        return np.ascontiguousarray(np.tile(w, (8, 1)))

    dkeys = np.arange(128, dtype=np.int64)
    per_core = []
    for c in range(NCORES):
        sl = slice(c * SLOTS, (c + 1) * SLOTS)
        srcidx = wrap16(SRC[sl])                      # [128, SLOTS/16] i16
        dcol = DCOL[sl].reshape(NW, K, 128)           # [w, k, p]
        # O[p, w, k, d] = 1 iff dst col of slot (w,k,p) == d
        oh = (dcol[:, :, :, None] == dkeys).astype(np.float32)  # [w,k,p,d]
        O = np.ascontiguousarray(oh.transpose(2, 0, 1, 3)).astype(BF)
        OT = np.ascontiguousarray(oh.transpose(3, 0, 1, 2)).astype(BF)
        nodes = c * SH + np.arange(SHP)
        gid = np.where(nodes < (c + 1) * SH, batch[np.minimum(nodes, N - 1)], -1)
        gidcol = gid.reshape(NW, 128).T.astype(np.float32)  # [128, NW]
        per_core.append(dict(srcidx=srcidx, O=O, OT=OT, gidcol=gidcol))
    cnts = np.bincount(batch.astype(np.int64), minlength=G).astype(np.float32)
    invcnt = (1.0 / np.maximum(cnts, 1.0)).reshape(G, 1)
    return K, per_core, invcnt


# ------------------------------------------------------------ device program
def build_program(K):
    nc = bacc.Bacc("TRN2", num_devices=NCORES)
    IDXW = NW * K * 128 // 16   # idx cols per core

    # ---- inputs
    xT0 = nc.dram_tensor("xT0", [P, 2, SHP], BF16, kind="ExternalInput")
    w1 = nc.dram_tensor("w1", [P, 2, 512], BF16, kind="ExternalInput")
    w2 = nc.dram_tensor("w2", [P, 4, 512], BF16, kind="ExternalInput")
    w3 = nc.dram_tensor("w3", [P, 4, 128], BF16, kind="ExternalInput")
    a1 = nc.dram_tensor("a1", [P, 8], BF16, kind="ExternalInput")
    a2 = nc.dram_tensor("a2", [P, 8], BF16, kind="ExternalInput")
    a3 = nc.dram_tensor("a3", [P, 2], BF16, kind="ExternalInput")
    srcidx_d = nc.dram_tensor("srcidx", [P, IDXW], I16, kind="ExternalInput")
    O_d = nc.dram_tensor("Omat", [P, NW, K, 128], BF16, kind="ExternalInput")
    OT_d = nc.dram_tensor("OTmat", [P, NW, K, 128], BF16, kind="ExternalInput")
    gidcol_d = nc.dram_tensor("gidcol", [P, NW], F32, kind="ExternalInput")
    iota64_d = nc.dram_tensor("iota64", [P, G], BF16, kind="ExternalInput")
    idbf_d = nc.dram_tensor("idbf", [P, P], BF16, kind="ExternalInput")
    idf32_d = nc.dram_tensor("idf32", [P, P], F32, kind="ExternalInput")
    invcnt_d = nc.dram_tensor("invcnt", [G, 1], F32, kind="ExternalInput")
    fcw_d = nc.dram_tensor("fcw", [HID, OUT_CH], F32, kind="ExternalInput")
    out_d = nc.dram_tensor("logits", [G, OUT_CH], F32, kind="ExternalOutput")
    hdump_d = nc.dram_tensor("hdump", [NP, ROW1], BF16, kind="ExternalOutput")
    xdump_d = nc.dram_tensor("xdump", [SHP, 512], BF16, kind="ExternalOutput")

    # ---- DRAM internals / collective buffers
    hag_in = [
        nc.dram_tensor(f"hag_in{l}", [SHP, ROW1 if l < 2 else ROW3], BF16)
        for l in range(3)
    ]
    hag_out = [
        nc.dram_tensor(
            f"hag_out{l}", [NP, ROW1 if l < 2 else ROW3], BF16,
            addr_space="Shared",
        )
        for l in range(3)
    ]
    xrows = [nc.dram_tensor(f"xrows{l}", [SHP, 512], BF16) for l in (1, 2)]
    ar_in = nc.dram_tensor("ar_in", [G, HID], F32)
    ar_out = nc.dram_tensor("ar_out", [G, HID], F32, addr_space="Shared")

    RG = [list(range(NCORES))]
    NT = SHP // 512  # 5 tiles per shard in the dense phase
    dma_sem = nc.alloc_semaphore("swdge_dma")

    with tile.TileContext(nc) as tc:
        with (
            tc.tile_pool(name="const", bufs=1) as cpool,
            tc.tile_pool(name="xbuf", bufs=1) as xpool,
            tc.tile_pool(name="dense", bufs=2) as dpool,
            tc.tile_pool(name="gather", bufs=2) as gpool,
            tc.tile_pool(name="work", bufs=2) as wpool,
            tc.tile_pool(name="psA", bufs=1, space="PSUM") as psA,
        ):
            # ---- constants into SBUF
            def load_const(dram, shape, dt, name):
                t = cpool.tile(shape, dt, name=name)
                nc.sync.dma_start(t[:], dram[:])
                return t

            iota64 = load_const(iota64_d, [P, G], BF16, "iota64")
            idbf = load_const(idbf_d, [P, P], BF16, "idbf")
            idf32 = load_const(idf32_d, [P, P], F32, "idf32")
            invcnt = load_const(invcnt_d, [G, 1], F32, "invcnt")
            fcw = load_const(fcw_d, [HID, OUT_CH], F32, "fcw")
            a_sb = [
                load_const(d, [P, n], BF16, f"a{i+1}")
                for i, (d, n) in enumerate([(a1, 8), (a2, 8), (a3, 2)])
            ]
            w_sb = [
                load_const(w1, [P, 2, 512], BF16, "w1"),
                load_const(w2, [P, 4, 512], BF16, "w2"),
                load_const(w3, [P, 4, 128], BF16, "w3"),
            ]
            srcidx = load_const(srcidx_d, [P, IDXW], I16, "srcidx")
            gidcol = load_const(gidcol_d, [P, NW], F32, "gidcol")
            zero1 = cpool.tile([P, 1], F32, name="zero1")
            nc.vector.memset(zero1[:], 0.0)

            # persistent SBUF buffers
            xT = xpool.tile([P, 4, SHP], BF16, name="xT")
            nc.sync.dma_start(xT[:, 0:2, :], xT0[:])
            ht = xpool.tile([P, 4, SHP], BF16, name="ht")
            adrec = xpool.tile([P, NW, HEADS], BF16, name="adrec")

            def dense_phase(l):
                """X^T -> H^T; alpha logits; row table -> hag_in; AllGather."""
                cinb = [2, 4, 4][l]
                coutb = [4, 4, 1][l]
                nh = [4, 4, 1][l]
                rowc = ROW1 if l < 2 else ROW3
                # H^T = W^T @ X^T
                for co in range(coutb):
                    for t in range(NT):
                        pm = psA.tile([P, 512], F32, tag=f"a{t % 2}",
                                      name=f"mm{l}_{co}_{t}")
                        for k in range(cinb):
                            nc.tensor.matmul(
                                out=pm[:],
                                lhsT=w_sb[l][:, k, co * 128: co * 128 + 128],
                                rhs=xT[:, k, t * 512: (t + 1) * 512],
                                start=(k == 0),
                                stop=(k == cinb - 1),
                            )
                        nc.vector.tensor_copy(
                            ht[:, co, t * 512: (t + 1) * 512], pm[:]
                        )
                # alpha logits [2, SHP] f32 per head
                ast_h = []
                for h in range(nh):
                    ah = dpool.tile([2, SHP], BF16, tag=f"ast{h}", bufs=1,
                                    name=f"ast{l}_{h}")
                    for t in range(NT):
                        pa = psA.tile([2, 512], F32, tag=f"c{t % 2}",
                                      name=f"aps{l}_{h}_{t}")
                        nc.tensor.matmul(
                            out=pa[:],
                            lhsT=a_sb[l][:, 2 * h: 2 * h + 2],
                            rhs=ht[:, h, t * 512: (t + 1) * 512],
                            start=True,
                            stop=True,
                        )
                        nc.vector.tensor_copy(ah[:, t * 512: (t + 1) * 512], pa[:])
                    ast_h.append(ah)
                # per-window: transpose H^T into interleaved rows + a-records
                for w in range(NW):
                    ws = slice(w * 128, (w + 1) * 128)
                    rows = dpool.tile([P, rowc], BF16, tag="rows",
                                      name=f"rows{l}_{w}")
                    rview = (
                        rows[:, 0:512].rearrange("p (c h) -> p c h", h=4)
                        if nh == 4 else rows[:, 0:128]
                    )
                    for co in range(coutb):
                        pt = psA.tile([P, P], BF16, tag=f"b{co % 2}",
                                      name=f"htp{l}_{w}_{co}")
                        nc.tensor.matmul(
                            out=pt[:], lhsT=ht[:, co, ws], rhs=idbf[:],
                            start=True, stop=True, is_transpose=True,
                        )
                        if nh == 4:
                            nc.vector.tensor_copy(rview[:, :, co], pt[:])
                        else:
                            nc.vector.tensor_copy(rview[:, :], pt[:])
                    # a_src / a_dst records: transpose [2,128] -> [128,2] bf16
                    for h in range(nh):
                        pr = psA.tile([P, 2], BF16, tag=f"c{h % 2}",
                                      name=f"arec{l}_{w}_{h}")
                        nc.tensor.matmul(
                            out=pr[:], lhsT=ast_h[h][:, ws], rhs=idbf[:2, :2],
                            start=True, stop=True, is_transpose=True,
                        )
                        nc.vector.tensor_copy(
                            rows[:, 512 + h: 513 + h] if l < 2
                            else rows[:, 128:129],
                            pr[:, 0:1],
                        )
                        nc.vector.tensor_copy(adrec[:, w, h: h + 1], pr[:, 1:2])
                    nc.sync.dma_start(hag_in[l][ws, :], rows[:])
                nc.gpsimd.collective_compute(
                    "AllGather", AluOp.bypass, replica_groups=RG,
                    ins=[hag_in[l][:]], outs=[hag_out[l][:]],
                )

            def agg_phase(l, pool_ps=None):
                """Gather + attention + scatter; rows out (elu'd)."""
                nh = [4, 4, 1][l]
                C = [512, 512, 128][l]
                rowc = ROW1 if l < 2 else ROW3
                NI = K * 128
                for w in range(NW):
                    isl = slice(w * (NI // 16), (w + 1) * (NI // 16))
                    hg = gpool.tile([P, K, rowc], BF16, tag="hg", name=f"hg{l}_{w}")
                    nc.gpsimd.dma_gather(
                        hg[:], hag_out[l][:], srcidx[:, isl], NI, NI, rowc,
                        single_packet=False,
                    )
                    Ow = gpool.tile([P, K, 128], BF16, tag="Ow", name=f"O{l}_{w}")
                    nc.sync.dma_start(Ow[:], O_d[:, w, :, :])
                    OTw = gpool.tile([P, K, 128], BF16, tag="OTw", name=f"OT{l}_{w}")
                    nc.sync.dma_start(OTw[:], OT_d[:, w, :, :])
                    # a_dst per edge via O^T @ ad_window  -> [128, K, nh] psum
                    adps = psA.tile([P, K * nh], F32, tag=f"b{w % 2}",
                                    name=f"adps{l}_{w}")
                    for k in range(K):
                        nc.tensor.matmul(
                            out=adps[:, k * nh: (k + 1) * nh],
                            lhsT=OTw[:, k, :], rhs=adrec[:, w, 0:nh],
                            start=True, stop=True,
                        )
                    # q = exp(lrelu(as + ad)) -> bf16 [128, K, nh]
                    asv = (
                        hg[:, :, 512:516] if l < 2 else hg[:, :, 128:129]
                    )  # [128, K, nh] bf16
                    tq = wpool.tile([P, K, nh], F32, tag="tq", name=f"tq{l}_{w}")
                    nc.vector.tensor_tensor(
                        out=tq[:], in0=asv,
                        in1=adps[:].rearrange("p (k h) -> p k h", h=nh),
                        op=AluOp.add,
                    )
                    ql = wpool.tile([P, K, nh], F32, tag="ql", name=f"ql{l}_{w}")
                    nc.vector.scalar_tensor_tensor(
                        out=ql[:], in0=tq[:], scalar=NEG_SLOPE, in1=tq[:],
                        op0=AluOp.mult, op1=AluOp.max,
                    )
                    qf = wpool.tile([P, K, nh], BF16, tag="qf", name=f"qf{l}_{w}")
                    nc.scalar.activation(qf[:], ql[:], Act.Exp)
                    if nh == 1:
                        qf32 = wpool.tile([P, K, 1], F32, tag="qf32",
                                          name=f"qf32{l}_{w}")
                        nc.scalar.activation(qf32[:], ql[:], Act.Exp)
                    # hgs = hg * q (broadcast over channels), per chunk
                    hgs = wpool.tile([P, K, C], BF16, tag="hgs", bufs=1, name=f"hgs{l}_{w}")
                    pagg = psA.tile([P, C], F32, tag=f"a{w % 2}", name=f"pagg{l}_{w}")
                    den = psA.tile([P, nh], F32, tag=f"c{w % 2}", name=f"den{l}_{w}")
                    for k in range(K):
                        if nh == 4:
                            nc.vector.tensor_tensor(
                                out=hgs[:, k, :].rearrange("p (c h) -> p c h", h=4),
                                in0=hg[:, k, 0:512].rearrange("p (c h) -> p c h", h=4),
                                in1=qf[:, k, :].unsqueeze(1).broadcast_to(
                                    [P, 128, 4]
                                ),
                                op=AluOp.mult,
                            )
                        else:
                            nc.vector.tensor_scalar(
                                out=hgs[:, k, :], in0=hg[:, k, 0:128],
                                scalar1=qf32[:, k, 0:1], scalar2=None,
                                op0=AluOp.mult,
                            )
                        nc.tensor.matmul(
                            out=pagg[:], lhsT=Ow[:, k, :], rhs=hgs[:, k, :],
                            start=(k == 0), stop=(k == K - 1),
                        )
                        nc.tensor.matmul(
                            out=den[:], lhsT=Ow[:, k, :], rhs=qf[:, k, :],
                            start=(k == 0), stop=(k == K - 1),
                        )
                    # normalize + elu -> rows (bf16)
                    rec = wpool.tile([P, nh], F32, tag="rec", name=f"rec{l}_{w}")
                    nc.vector.tensor_scalar(
                        out=rec[:], in0=den[:], scalar1=1e-16, scalar2=None,
                        op0=AluOp.add,
                    )
                    nc.vector.reciprocal(rec[:], rec[:])
                    tmul = wpool.tile([P, C], F32, tag="tmul", bufs=1, name=f"tm{l}_{w}")
                    if nh == 4:
                        nc.vector.tensor_tensor(
                            out=tmul[:].rearrange("p (c h) -> p c h", h=4),
                            in0=pagg[:].rearrange("p (c h) -> p c h", h=4),
                            in1=rec[:].unsqueeze(1).broadcast_to([P, 128, 4]),
                            op=AluOp.mult,
                        )
                    else:
                        nc.vector.tensor_scalar(
                            out=tmul[:], in0=pagg[:], scalar1=rec[:, 0:1],
                            scalar2=None, op0=AluOp.mult,
                        )
                    tmin = wpool.tile([P, C], F32, tag="tmin", bufs=1, name=f"tn{l}_{w}")
                    nc.vector.tensor_scalar(
                        out=tmin[:], in0=tmul[:], scalar1=0.0, scalar2=None,
                        op0=AluOp.min,
                    )
                    em = wpool.tile([P, C], F32, tag="em", bufs=1, name=f"em{l}_{w}")
                    nc.scalar.activation(em[:], tmin[:], Act.Exp)
                    relu = wpool.tile([P, C], F32, tag="relu", bufs=1, name=f"rl{l}_{w}")
                    nc.vector.tensor_scalar(
                        out=relu[:], in0=tmul[:], scalar1=0.0, scalar2=None,
                        op0=AluOp.max,
                    )
                    orow = wpool.tile([P, C], BF16, tag="orow", name=f"or{l}_{w}")
                    nc.vector.scalar_tensor_tensor(
                        out=orow[:], in0=em[:], scalar=-1.0, in1=relu[:],
                        op0=AluOp.add, op1=AluOp.add,
                    )
                    if l < 2:
                        nc.sync.dma_start(xrows[l][w * 128:(w + 1) * 128, :], orow[:])
                    else:
                        # fuse graph pooling: pool_ps += gsel^T @ rows
                        gw = wpool.tile([P, G], BF16, tag="gw", name=f"gw_{w}")
                        nc.vector.tensor_scalar(
                            out=gw[:], in0=iota64[:], scalar1=gidcol[:, w: w + 1],
                            scalar2=None, op0=AluOp.is_equal,
                        )
                        nc.tensor.matmul(
                            out=pool_ps[:], lhsT=gw[:], rhs=orow[:],
                            start=(w == 0), stop=(w == NW - 1),
                        )

            def load_xT(l):
                """X^T for layer l in {1,2} via HWDGE dma-transpose of rows."""
                for b in range(4):
                    nc.sync.dma_start_transpose(
                        xT[:, b, :], xrows[l - 1][:, b * 128:(b + 1) * 128]
                    )

            def pool_fc(pool_ps):
                psums = wpool.tile([G, HID], F32, tag="psums", name="psums")
                nc.vector.tensor_copy(psums[:], pool_ps[:])
                nc.sync.dma_start(ar_in[:], psums[:])
                nc.gpsimd.collective_compute(
                    "AllReduce", AluOp.add, replica_groups=RG,
                    ins=[ar_in[:]], outs=[ar_out[:]],
                )
                sums = wpool.tile([G, HID], F32, tag="sums", name="sums")
                nc.sync.dma_start(sums[:], ar_out[:])
                pooled = wpool.tile([G, HID], F32, tag="pooled", name="pooled")
                nc.vector.tensor_scalar(
                    out=pooled[:], in0=sums[:], scalar1=invcnt[:, 0:1],
                    scalar2=None, op0=AluOp.mult,
                )
                ptp = psA.tile([HID, G], F32, tag="c0", name="poolT")
                nc.tensor.matmul(
                    out=ptp[:], lhsT=pooled[:], rhs=idf32[:G, :G],
                    start=True, stop=True, is_transpose=True,
                )
                poolT = wpool.tile([HID, G], F32, tag="poolT", name="poolTs")
                nc.vector.tensor_copy(poolT[:], ptp[:])
                pfc = psA.tile([G, OUT_CH], F32, tag="b0", name="fcps")
                nc.tensor.matmul(
                    out=pfc[:], lhsT=poolT[:], rhs=fcw[:], start=True, stop=True
                )
                logits = wpool.tile([G, OUT_CH], F32, tag="logits", name="logits")
                nc.vector.tensor_copy(logits[:], pfc[:])
                nc.sync.dma_start(out_d[:], logits[:])

            dense_phase(0)
            agg_phase(0)
            load_xT(1)
            dense_phase(1)
            agg_phase(1)
            load_xT(2)
            dense_phase(2)
            pool_ps = psA.tile([G, HID], F32, tag="d0", name="poolps")
            agg_phase(2, pool_ps)
            pool_fc(pool_ps)

            if os.environ.get("DUMP_H"):
                li = int(os.environ["DUMP_H"])
                cw = ROW1 if li < 2 else ROW3
                hstg = wpool.tile([P, cw], BF16, tag="hdmp", bufs=2, name="hdmp")
                for b in range(NP // P):
                    lo, hi = b * P, (b + 1) * P
                    nc.sync.dma_start(hstg[:], hag_out[li][lo:hi, :])
                    nc.sync.dma_start(hdump_d[lo:hi, 0:cw], hstg[:])
            if os.environ.get("DUMP_X"):
                xi = int(os.environ["DUMP_X"])  # 1 or 2: xrows after agg xi-1
                xstg = wpool.tile([P, 512], BF16, tag="xdmp", bufs=2, name="xdmp")
                for b in range(SHP // P):
                    lo, hi = b * P, (b + 1) * P
                    nc.sync.dma_start(xstg[:], xrows[xi - 1][lo:hi, :])
                    nc.sync.dma_start(xdump_d[lo:hi, :], xstg[:])

    nc.compile()
    return nc


_prog_cache = {}


def _interleave_perm():
    # perm[j] = flat channel index stored at interleaved col j
    j = np.arange(512)
    c, h = j // 4, j % 4
    return h * 128 + c


def kernel(x, edge_index, batch, W1, a_src1, a_dst1, b1,
           W2, a_src2, a_dst2, b2, W3, a_src3, a_dst3, b3, fc_w, fc_b,
           _want_results=False, _trace=False):
    x = np.asarray(x)
    edge_index = np.asarray(edge_index)
    batch = np.asarray(batch)
    for b in (b1, b2, b3, fc_b):
        assert not np.any(np.asarray(b)), "nonzero biases not supported"

    K, per_core, invcnt = preprocess(edge_index, batch)
    if K not in _prog_cache:
        _prog_cache[K] = build_program(K)
    nc = _prog_cache[K]

    iota64 = np.ascontiguousarray(
        np.broadcast_to(np.arange(G, dtype=np.float32), (P, G)).astype(BF)
    )
    idbf = np.eye(P, dtype=np.float32).astype(BF)
    idf32 = np.eye(P, dtype=np.float32)
    perm = _interleave_perm()

    def wmat(W, cinb, cout, perm_in=None):
        Wf = np.asarray(W, np.float32)
        if perm_in is not None:
            Wf = Wf[perm_in]
        return np.ascontiguousarray(
            Wf.reshape(cinb, 128, cout).transpose(1, 0, 2)
        ).astype(BF)

    w1m = wmat(W1, 2, 512)
    w2m = wmat(W2, 4, 512, perm)
    w3m = wmat(W3, 4, 128, perm)

    def avec(asrc, adst):
        nh = asrc.shape[0]
        out = np.empty((128, 2 * nh), np.float32)
        out[:, 0::2] = np.asarray(asrc, np.float32).T
        out[:, 1::2] = np.asarray(adst, np.float32).T
        return np.ascontiguousarray(out).astype(BF)

    a1m = avec(a_src1, a_dst1)
    a2m = avec(a_src2, a_dst2)
    a3m = avec(a_src3, a_dst3)
    fcw = np.ascontiguousarray(np.asarray(fc_w, np.float32))

    xf = np.asarray(x, np.float32)
    in_maps = []
    for c in range(NCORES):
        xs = np.zeros((IN_CH, SHP), np.float32)
        xs[:, :SH] = xf[c * SH: (c + 1) * SH].T
        pc = per_core[c]
        in_maps.append(
            dict(
                xT0=np.ascontiguousarray(
                    xs.reshape(2, 128, SHP).transpose(1, 0, 2)
                ).astype(BF),
                w1=w1m, w2=w2m, w3=w3m, a1=a1m, a2=a2m, a3=a3m,
                srcidx=pc["srcidx"], Omat=pc["O"], OTmat=pc["OT"],
                gidcol=pc["gidcol"],
                iota64=iota64, idbf=idbf, idf32=idf32, invcnt=invcnt, fcw=fcw,
            )
        )
    res = run_bass_kernel_spmd(
        nc, in_maps, list(range(NCORES)), trace=_trace
    )
    out = res.results[0]["logits"].astype(np.float32)
    if _want_results:
        return out, res
    return out


# revision 11
# speedup vs baseline: 2.2931x; 1.1088x over previous
"""GAT (3-layer, 4-head) + global mean pool + FC on 8 Trainium2 NeuronCores.

Strategy (v2)
-------------
Nodes sharded contiguously across 8 cores (2500 each, padded to 2560; `batch`
is sorted so this is graph-aligned data parallelism). Per layer:
  1. Dense phase: H^T = W^T X^T on the PE; attention logits a_src/a_dst per
     node; H^T transposed into a per-node row table [h(interleaved)|a_src]
     and AllGathered so every core holds the full 20480-row table in HBM.
  2. Agg phase, per 128-dst window: ONE dma_gather fetches h+a_src rows for
     the window's edges (sorted by dst, chunked 128/chunk). Host-precomputed
     one-hot scatter matrices O / O^T (static edge structure) stream in via
     HWDGE. a_dst per edge = O^T @ a_dst_window on the PE; q = exp(lrelu(
     a_s+a_d)) on ACT; q folded into gathered rows with a single broadcast
     multiply per chunk (head-interleaved channels); PE contracts
     out[dst,c] += O^T(edges->dst) . (q*h) plus denominators. Normalize +
     ELU fused on full-width [128,512] ops with per-partition reciprocals.
  3. Next layer's X^T obtained by HWDGE dma-transpose of the row output.
Final: graph mean-pool fused into layer-3 agg windows, AllReduce, FC.
"""
import os
import sys

sys.path.insert(0, "/opt/trn_rl_repo")

import ml_dtypes
import numpy as np

import concourse.bass as bass
import concourse.tile as tile
from concourse import bacc, mybir
from concourse.bass_utils import run_bass_kernel_spmd

# problem constants (hardcoded per the harness contract)
N = 20000
E0 = 320000
IN_CH = 256
HID = 128
HEADS = 4
OUT_CH = 200
G = 64
NEG_SLOPE = 0.2
NCORES = 8
SH = N // NCORES          # 2500 nodes per core
NW = (SH + 127) // 128    # 20 windows per core
SHP = NW * 128            # 2560 padded shard
NP = NCORES * SHP         # 20480 padded global rows
P = 128
ROW1 = 640                # table row cols (bf16) for layers 1-2: 512 h + 8 as-f32 + pad
ROW3 = 256                # layer 3: 128 h + 2 as-f32 + pad

F32 = mybir.dt.float32
BF16 = mybir.dt.bfloat16
I16 = mybir.dt.int16
BF = ml_dtypes.bfloat16

AluOp = mybir.AluOpType
Act = mybir.ActivationFunctionType


# ----------------------------------------------------------------- host prep
def preprocess(edge_index, batch):
    src = np.concatenate([edge_index[0].astype(np.int64), np.arange(N)])
    dst = np.concatenate([edge_index[1].astype(np.int64), np.arange(N)])
    order = np.argsort(dst, kind="stable")
    src_s = src[order]
    dst_s = dst[order]

    core = dst_s // SH
    win = (dst_s % SH) // 128
    group = core * NW + win                      # 0..159, nondecreasing
    counts = np.bincount(group, minlength=NCORES * NW)
    K = int(np.ceil(counts.max() / 128))
    SLOTS = NW * K * 128

    starts = np.zeros(NCORES * NW, np.int64)
    starts[1:] = np.cumsum(counts)[:-1]
    rank = np.arange(len(dst_s)) - starts[group]
    slot = group * (K * 128) + rank              # global slot id

    SRC = np.zeros(NCORES * SLOTS, np.int64)
    DCOL = np.full(NCORES * SLOTS, -1, np.int64)
    # remap src node id into the padded 2560-per-core row space
    SRC[slot] = (src_s // SH) * SHP + (src_s % SH)
    DCOL[slot] = dst_s - core * SH - win * 128

    def wrap16(a):
        # slot i -> [i%16, i//16], replicated to 128 partitions
        w = a.reshape(-1, 16).T.astype(np.int16)     # [16, SLOTS/16]
        return np.ascontiguousarray(np.tile(w, (8, 1)))

    dkeys = np.arange(128, dtype=np.int64)
    per_core = []
    for c in range(NCORES):
        sl = slice(c * SLOTS, (c + 1) * SLOTS)
        srcidx = wrap16(SRC[sl])                      # [128, SLOTS/16] i16
        dcol = DCOL[sl].reshape(NW, K, 128)           # [w, k, p]
        # O[p, w, k, d] = 1 iff dst col of slot (w,k,p) == d
        oh = (dcol[:, :, :, None] == dkeys).astype(np.float32)  # [w,k,p,d]
        O = np.ascontiguousarray(oh.transpose(2, 0, 1, 3)).astype(BF)
        OT = np.ascontiguousarray(oh.transpose(3, 0, 1, 2)).astype(BF)
        nodes = c * SH + np.arange(SHP)
        gid = np.where(nodes < (c + 1) * SH, batch[np.minimum(nodes, N - 1)], -1)
        gidcol = gid.reshape(NW, 128).T.astype(np.float32)  # [128, NW]
        per_core.append(dict(srcidx=srcidx, O=O, OT=OT, gidcol=gidcol))
    cnts = np.bincount(batch.astype(np.int64), minlength=G).astype(np.float32)
    invcnt = (1.0 / np.maximum(cnts, 1.0)).reshape(G, 1)
    return K, per_core, invcnt


# ------------------------------------------------------------ device program
def build_program(K):
    nc = bacc.Bacc("TRN2", num_devices=NCORES)
    IDXW = NW * K * 128 // 16   # idx cols per core

    # ---- inputs
    xT0 = nc.dram_tensor("xT0", [P, 2, SHP], BF16, kind="ExternalInput")
    w1 = nc.dram_tensor("w1", [P, 2, 512], BF16, kind="ExternalInput")
    w2 = nc.dram_tensor("w2", [P, 4, 512], BF16, kind="ExternalInput")
    w3 = nc.dram_tensor("w3", [P, 4, 128], BF16, kind="ExternalInput")
    a1 = nc.dram_tensor("a1", [P, 8], BF16, kind="ExternalInput")
    a2 = nc.dram_tensor("a2", [P, 8], BF16, kind="ExternalInput")
    a3 = nc.dram_tensor("a3", [P, 2], BF16, kind="ExternalInput")
    srcidx_d = nc.dram_tensor("srcidx", [P, IDXW], I16, kind="ExternalInput")
    O_d = nc.dram_tensor("Omat", [P, NW, K, 128], BF16, kind="ExternalInput")
    OT_d = nc.dram_tensor("OTmat", [P, NW, K, 128], BF16, kind="ExternalInput")
    gidcol_d = nc.dram_tensor("gidcol", [P, NW], F32, kind="ExternalInput")
    iota64_d = nc.dram_tensor("iota64", [P, G], BF16, kind="ExternalInput")
    idbf_d = nc.dram_tensor("idbf", [P, P], BF16, kind="ExternalInput")
    idf32_d = nc.dram_tensor("idf32", [P, P], F32, kind="ExternalInput")
    invcnt_d = nc.dram_tensor("invcnt", [G, 1], F32, kind="ExternalInput")
    fcw_d = nc.dram_tensor("fcw", [HID, OUT_CH], F32, kind="ExternalInput")
    out_d = nc.dram_tensor("logits", [G, OUT_CH], F32, kind="ExternalOutput")
    hdump_d = nc.dram_tensor("hdump", [NP, ROW1], BF16, kind="ExternalOutput")
    xdump_d = nc.dram_tensor("xdump", [SHP, 512], BF16, kind="ExternalOutput")

    # ---- DRAM internals / collective buffers
    hag_in = [
        nc.dram_tensor(f"hag_in{l}", [SHP, ROW1 if l < 2 else ROW3], BF16)
        for l in range(3)
    ]
    hag_out = [
        nc.dram_tensor(
            f"hag_out{l}", [NP, ROW1 if l < 2 else ROW3], BF16,
            addr_space="Shared",
        )
        for l in range(3)
    ]
    xrows = [nc.dram_tensor(f"xrows{l}", [SHP, 512], BF16) for l in (1, 2)]
    ar_in = nc.dram_tensor("ar_in", [G, HID], F32)
    ar_out = nc.dram_tensor("ar_out", [G, HID], F32, addr_space="Shared")

    RG = [list(range(NCORES))]
    NT = SHP // 512  # 5 tiles per shard in the dense phase
    dma_sem = nc.alloc_semaphore("swdge_dma")

    with tile.TileContext(nc) as tc:
        with (
            tc.tile_pool(name="const", bufs=1) as cpool,
            tc.tile_pool(name="xbuf", bufs=1) as xpool,
            tc.tile_pool(name="dense", bufs=2) as dpool,
            tc.tile_pool(name="gather", bufs=2) as gpool,
            tc.tile_pool(name="work", bufs=2) as wpool,
            tc.tile_pool(name="psA", bufs=1, space="PSUM") as psA,
        ):
            # ---- constants into SBUF
            def load_const(dram, shape, dt, name):
                t = cpool.tile(shape, dt, name=name)
                nc.sync.dma_start(t[:], dram[:])
                return t

            iota64 = load_const(iota64_d, [P, G], BF16, "iota64")
            idbf = load_const(idbf_d, [P, P], BF16, "idbf")
            idf32 = load_const(idf32_d, [P, P], F32, "idf32")
            invcnt = load_const(invcnt_d, [G, 1], F32, "invcnt")
            fcw = load_const(fcw_d, [HID, OUT_CH], F32, "fcw")
            a_sb = [
                load_const(d, [P, n], BF16, f"a{i+1}")
                for i, (d, n) in enumerate([(a1, 8), (a2, 8), (a3, 2)])
            ]
            w_sb = [
                load_const(w1, [P, 2, 512], BF16, "w1"),
                load_const(w2, [P, 4, 512], BF16, "w2"),
                load_const(w3, [P, 4, 128], BF16, "w3"),
            ]
            srcidx = load_const(srcidx_d, [P, IDXW], I16, "srcidx")
            gidcol = load_const(gidcol_d, [P, NW], F32, "gidcol")
            zero1 = cpool.tile([P, 1], F32, name="zero1")
            nc.vector.memset(zero1[:], 0.0)

            # persistent SBUF buffers
            xT = xpool.tile([P, 4, SHP], BF16, name="xT")
            nc.sync.dma_start(xT[:, 0:2, :], xT0[:])
            ht = xpool.tile([P, 4, SHP], BF16, name="ht")
            adrec = xpool.tile([P, NW, HEADS], BF16, name="adrec")

            def dense_phase(l):
                """X^T -> H^T; alpha logits; row table -> hag_in; AllGather."""
                cinb = [2, 4, 4][l]
                coutb = [4, 4, 1][l]
                nh = [4, 4, 1][l]
                rowc = ROW1 if l < 2 else ROW3
                # H^T = W^T @ X^T
                for co in range(coutb):
                    for t in range(NT):
                        pm = psA.tile([P, 512], F32, tag=f"a{t % 2}",
                                      name=f"mm{l}_{co}_{t}")
                        for k in range(cinb):
                            nc.tensor.matmul(
                                out=pm[:],
                                lhsT=w_sb[l][:, k, co * 128: co * 128 + 128],
                                rhs=xT[:, k, t * 512: (t + 1) * 512],
                                start=(k == 0),
                                stop=(k == cinb - 1),
                            )
                        nc.vector.tensor_copy(
                            ht[:, co, t * 512: (t + 1) * 512], pm[:]
                        )
                # alpha logits [2, SHP] f32 per head
                ast_h = []
                for h in range(nh):
                    ah = dpool.tile([2, SHP], BF16, tag=f"ast{h}", bufs=1,
                                    name=f"ast{l}_{h}")
                    for t in range(NT):
                        pa = psA.tile([2, 512], F32, tag=f"c{t % 2}",
                                      name=f"aps{l}_{h}_{t}")
                        nc.tensor.matmul(
                            out=pa[:],
                            lhsT=a_sb[l][:, 2 * h: 2 * h + 2],
                            rhs=ht[:, h, t * 512: (t + 1) * 512],
                            start=True,
                            stop=True,
                        )
                        nc.vector.tensor_copy(ah[:, t * 512: (t + 1) * 512], pa[:])
                    ast_h.append(ah)
                # per-window: transpose H^T into interleaved rows + a-records
                for w in range(NW):
                    ws = slice(w * 128, (w + 1) * 128)
                    rows = dpool.tile([P, rowc], BF16, tag="rows",
                                      name=f"rows{l}_{w}")
                    rview = (
                        rows[:, 0:512].rearrange("p (c h) -> p c h", h=4)
                        if nh == 4 else rows[:, 0:128]
                    )
                    for co in range(coutb):
                        pt = psA.tile([P, P], BF16, tag=f"b{co % 2}",
                                      name=f"htp{l}_{w}_{co}")
                        nc.tensor.matmul(
                            out=pt[:], lhsT=ht[:, co, ws], rhs=idbf[:],
                            start=True, stop=True, is_transpose=True,
                        )
                        if nh == 4:
                            nc.vector.tensor_copy(rview[:, :, co], pt[:])
                        else:
                            nc.vector.tensor_copy(rview[:, :], pt[:])
                    # a_src / a_dst records: transpose [2,128] -> [128,2] bf16
                    for h in range(nh):
                        pr = psA.tile([P, 2], BF16, tag=f"c{h % 2}",
                                      name=f"arec{l}_{w}_{h}")
                        nc.tensor.matmul(
                            out=pr[:], lhsT=ast_h[h][:, ws], rhs=idbf[:2, :2],
                            start=True, stop=True, is_transpose=True,
                        )
                        nc.vector.tensor_copy(
                            rows[:, 512 + h: 513 + h] if l < 2
                            else rows[:, 128:129],
                            pr[:, 0:1],
                        )
                        nc.vector.tensor_copy(adrec[:, w, h: h + 1], pr[:, 1:2])
                    nc.sync.dma_start(hag_in[l][ws, :], rows[:])
                nc.gpsimd.collective_compute(
                    "AllGather", AluOp.bypass, replica_groups=RG,
                    ins=[hag_in[l][:]], outs=[hag_out[l][:]],
                )

            def agg_phase(l, pool_ps=None):
                """Gather + attention + scatter; rows out (elu'd)."""
                nh = [4, 4, 1][l]
                C = [512, 512, 128][l]
                rowc = ROW1 if l < 2 else ROW3
                NI = K * 128
                for w in range(NW):
                    isl = slice(w * (NI // 16), (w + 1) * (NI // 16))
                    hg = gpool.tile([P, K, rowc], BF16, tag="hg", name=f"hg{l}_{w}")
                    nc.gpsimd.dma_gather(
                        hg[:], hag_out[l][:], srcidx[:, isl], NI, NI, rowc,
                        single_packet=False,
                    )
                    Ow = gpool.tile([P, K, 128], BF16, tag="Ow", name=f"O{l}_{w}")
                    nc.sync.dma_start(Ow[:], O_d[:, w, :, :])
                    OTw = gpool.tile([P, K, 128], BF16, tag="OTw", name=f"OT{l}_{w}")
                    nc.sync.dma_start(OTw[:], OT_d[:, w, :, :])
                    # a_dst per edge via O^T @ ad_window  -> [128, K, nh] psum
                    adps = psA.tile([P, K * nh], F32, tag=f"b{w % 2}",
                                    name=f"adps{l}_{w}")
                    for k in range(K):
                        nc.tensor.matmul(
                            out=adps[:, k * nh: (k + 1) * nh],
                            lhsT=OTw[:, k, :], rhs=adrec[:, w, 0:nh],
                            start=True, stop=True,
                        )
                    # q = exp(lrelu(as + ad)) -> bf16 [128, K, nh]
                    asv = (
                        hg[:, :, 512:516] if l < 2 else hg[:, :, 128:129]
                    )  # [128, K, nh] bf16
                    tq = wpool.tile([P, K, nh], F32, tag="tq", name=f"tq{l}_{w}")
                    nc.vector.tensor_tensor(
                        out=tq[:], in0=asv,
                        in1=adps[:].rearrange("p (k h) -> p k h", h=nh),
                        op=AluOp.add,
                    )
                    ql = wpool.tile([P, K, nh], F32, tag="ql", name=f"ql{l}_{w}")
                    nc.vector.scalar_tensor_tensor(
                        out=ql[:], in0=tq[:], scalar=NEG_SLOPE, in1=tq[:],
                        op0=AluOp.mult, op1=AluOp.max,
                    )
                    qf = wpool.tile([P, K, nh], BF16, tag="qf", name=f"qf{l}_{w}")
                    nc.scalar.activation(qf[:], ql[:], Act.Exp)
                    if nh == 1:
                        qf32 = wpool.tile([P, K, 1], F32, tag="qf32",
                                          name=f"qf32{l}_{w}")
                        nc.scalar.activation(qf32[:], ql[:], Act.Exp)
                    # hgs = hg * q (broadcast over channels), per chunk
                    hgs = wpool.tile([P, K, C], BF16, tag="hgs", bufs=1, name=f"hgs{l}_{w}")
                    pagg = psA.tile([P, C], F32, tag=f"a{w % 2}", name=f"pagg{l}_{w}")
                    den = psA.tile([P, nh], F32, tag=f"c{w % 2}", name=f"den{l}_{w}")
                    for k in range(K):
                        if nh == 4:
                            nc.vector.tensor_tensor(
                                out=hgs[:, k, :].rearrange("p (c h) -> p c h", h=4),
                                in0=hg[:, k, 0:512].rearrange("p (c h) -> p c h", h=4),
                                in1=qf[:, k, :].unsqueeze(1).broadcast_to(
                                    [P, 128, 4]
                                ),
                                op=AluOp.mult,
                            )
                        else:
                            nc.vector.tensor_tensor(
                                out=hgs[:, k, :], in0=hg[:, k, 0:128],
                                in1=qf32[:, k, 0:1].broadcast_to([P, 128]),
                                op=AluOp.mult,
                            )
                        nc.tensor.matmul(
                            out=pagg[:], lhsT=Ow[:, k, :], rhs=hgs[:, k, :],
                            start=(k == 0), stop=(k == K - 1),
                        )
                        nc.tensor.matmul(
                            out=den[:], lhsT=Ow[:, k, :], rhs=qf[:, k, :],
                            start=(k == 0), stop=(k == K - 1),
                        )
                    # normalize + elu -> rows (bf16)
                    rec = wpool.tile([P, nh], F32, tag="rec", name=f"rec{l}_{w}")
                    nc.vector.scalar_tensor_tensor(
                        out=rec[:], in0=den[:], scalar=1e-16, in1=zero1[:, 0:1].broadcast_to([P, nh]),
                        op0=AluOp.add, op1=AluOp.add,
                    )
                    nc.vector.reciprocal(rec[:], rec[:])
                    tmul = wpool.tile([P, C], F32, tag="tmul", bufs=1, name=f"tm{l}_{w}")
                    if nh == 4:
                        nc.vector.tensor_tensor(
                            out=tmul[:].rearrange("p (c h) -> p c h", h=4),
                            in0=pagg[:].rearrange("p (c h) -> p c h", h=4),
                            in1=rec[:].unsqueeze(1).broadcast_to([P, 128, 4]),
                            op=AluOp.mult,
                        )
                    else:
                        nc.vector.tensor_tensor(
                            out=tmul[:], in0=pagg[:],
                            in1=rec[:, 0:1].broadcast_to([P, 128]),
                            op=AluOp.mult,
                        )
                    tmin = wpool.tile([P, C], F32, tag="tmin", bufs=1, name=f"tn{l}_{w}")
                    nc.vector.scalar_tensor_tensor(
                        out=tmin[:], in0=tmul[:], scalar=0.0,
                        in1=zero1[:, 0:1].broadcast_to([P, C]),
                        op0=AluOp.add, op1=AluOp.min,
                    )
                    em = wpool.tile([P, C], F32, tag="em", bufs=1, name=f"em{l}_{w}")
                    nc.scalar.activation(em[:], tmin[:], Act.Exp)
                    relu = wpool.tile([P, C], F32, tag="relu", bufs=1, name=f"rl{l}_{w}")
                    nc.vector.tensor_scalar(
                        out=relu[:], in0=tmul[:], scalar1=0.0, scalar2=None,
                        op0=AluOp.max,
                    )
                    orow = wpool.tile([P, C], BF16, tag="orow", name=f"or{l}_{w}")
                    nc.vector.scalar_tensor_tensor(
                        out=orow[:], in0=em[:], scalar=-1.0, in1=relu[:],
                        op0=AluOp.add, op1=AluOp.add,
                    )
                    if l < 2:
                        nc.sync.dma_start(xrows[l][w * 128:(w + 1) * 128, :], orow[:])
                    else:
                        # fuse graph pooling: pool_ps += gsel^T @ rows
                        gw = wpool.tile([P, G], BF16, tag="gw", name=f"gw_{w}")
                        nc.vector.tensor_tensor(
                            out=gw[:], in0=iota64[:],
                            in1=gidcol[:, w: w + 1].broadcast_to([P, G]),
                            op=AluOp.is_equal,
                        )
                        nc.tensor.matmul(
                            out=pool_ps[:], lhsT=gw[:], rhs=orow[:],
                            start=(w == 0), stop=(w == NW - 1),
                        )

            def load_xT(l):
                """X^T for layer l in {1,2} via HWDGE dma-transpose of rows."""
                for b in range(4):
                    nc.sync.dma_start_transpose(
                        xT[:, b, :], xrows[l - 1][:, b * 128:(b + 1) * 128]
                    )

            def pool_fc(pool_ps):
                psums = wpool.tile([G, HID], F32, tag="psums", name="psums")
                nc.vector.tensor_copy(psums[:], pool_ps[:])
                nc.sync.dma_start(ar_in[:], psums[:])
                nc.gpsimd.collective_compute(
                    "AllReduce", AluOp.add, replica_groups=RG,
                    ins=[ar_in[:]], outs=[ar_out[:]],
                )
                sums = wpool.tile([G, HID], F32, tag="sums", name="sums")
                nc.sync.dma_start(sums[:], ar_out[:])
                pooled = wpool.tile([G, HID], F32, tag="pooled", name="pooled")
                nc.vector.tensor_scalar(
                    out=pooled[:], in0=sums[:], scalar1=invcnt[:, 0:1],
                    scalar2=None, op0=AluOp.mult,
                )
                ptp = psA.tile([HID, G], F32, tag="c0", name="poolT")
                nc.tensor.matmul(
                    out=ptp[:], lhsT=pooled[:], rhs=idf32[:G, :G],
                    start=True, stop=True, is_transpose=True,
                )
                poolT = wpool.tile([HID, G], F32, tag="poolT", name="poolTs")
                nc.vector.tensor_copy(poolT[:], ptp[:])
                pfc = psA.tile([G, OUT_CH], F32, tag="b0", name="fcps")
                nc.tensor.matmul(
                    out=pfc[:], lhsT=poolT[:], rhs=fcw[:], start=True, stop=True
                )
                logits = wpool.tile([G, OUT_CH], F32, tag="logits", name="logits")
                nc.vector.tensor_copy(logits[:], pfc[:])
                nc.sync.dma_start(out_d[:], logits[:])

            dense_phase(0)
            agg_phase(0)
            load_xT(1)
            dense_phase(1)
            agg_phase(1)
            load_xT(2)
            dense_phase(2)
            pool_ps = psA.tile([G, HID], F32, tag="d0", name="poolps")
            agg_phase(2, pool_ps)
            pool_fc(pool_ps)

            if os.environ.get("DUMP_H"):
                li = int(os.environ["DUMP_H"])
                cw = ROW1 if li < 2 else ROW3
                hstg = wpool.tile([P, cw], BF16, tag="hdmp", bufs=2, name="hdmp")
                for b in range(NP // P):
                    lo, hi = b * P, (b + 1) * P
                    nc.sync.dma_start(hstg[:], hag_out[li][lo:hi, :])
                    nc.sync.dma_start(hdump_d[lo:hi, 0:cw], hstg[:])
            if os.environ.get("DUMP_X"):
                xi = int(os.environ["DUMP_X"])  # 1 or 2: xrows after agg xi-1
                xstg = wpool.tile([P, 512], BF16, tag="xdmp", bufs=2, name="xdmp")
                for b in range(SHP // P):
                    lo, hi = b * P, (b + 1) * P
                    nc.sync.dma_start(xstg[:], xrows[xi - 1][lo:hi, :])
                    nc.sync.dma_start(xdump_d[lo:hi, :], xstg[:])

    nc.compile()
    return nc


_prog_cache = {}


def _interleave_perm():
    # perm[j] = flat channel index stored at interleaved col j
    j = np.arange(512)
    c, h = j // 4, j % 4
    return h * 128 + c


def kernel(x, edge_index, batch, W1, a_src1, a_dst1, b1,
           W2, a_src2, a_dst2, b2, W3, a_src3, a_dst3, b3, fc_w, fc_b,
           _want_results=False, _trace=False):
    x = np.asarray(x)
    edge_index = np.asarray(edge_index)
    batch = np.asarray(batch)
    for b in (b1, b2, b3, fc_b):
        assert not np.any(np.asarray(b)), "nonzero biases not supported"

    K, per_core, invcnt = preprocess(edge_index, batch)
    if K not in _prog_cache:
        _prog_cache[K] = build_program(K)
    nc = _prog_cache[K]

    iota64 = np.ascontiguousarray(
        np.broadcast_to(np.arange(G, dtype=np.float32), (P, G)).astype(BF)
    )
    idbf = np.eye(P, dtype=np.float32).astype(BF)
    idf32 = np.eye(P, dtype=np.float32)
    perm = _interleave_perm()

    def wmat(W, cinb, cout, perm_in=None):
        Wf = np.asarray(W, np.float32)
        if perm_in is not None:
            Wf = Wf[perm_in]
        return np.ascontiguousarray(
            Wf.reshape(cinb, 128, cout).transpose(1, 0, 2)
        ).astype(BF)

    w1m = wmat(W1, 2, 512)
    w2m = wmat(W2, 4, 512, perm)
    w3m = wmat(W3, 4, 128, perm)

    def avec(asrc, adst):
        nh = asrc.shape[0]
        out = np.empty((128, 2 * nh), np.float32)
        out[:, 0::2] = np.asarray(asrc, np.float32).T
        out[:, 1::2] = np.asarray(adst, np.float32).T
        return np.ascontiguousarray(out).astype(BF)

    a1m = avec(a_src1, a_dst1)
    a2m = avec(a_src2, a_dst2)
    a3m = avec(a_src3, a_dst3)
    fcw = np.ascontiguousarray(np.asarray(fc_w, np.float32))

    xf = np.asarray(x, np.float32)
    in_maps = []
    for c in range(NCORES):
        xs = np.zeros((IN_CH, SHP), np.float32)
        xs[:, :SH] = xf[c * SH: (c + 1) * SH].T
        pc = per_core[c]
        in_maps.append(
            dict(
                xT0=np.ascontiguousarray(
                    xs.reshape(2, 128, SHP).transpose(1, 0, 2)
                ).astype(BF),
                w1=w1m, w2=w2m, w3=w3m, a1=a1m, a2=a2m, a3=a3m,
                srcidx=pc["srcidx"], Omat=pc["O"], OTmat=pc["OT"],
                gidcol=pc["gidcol"],
                iota64=iota64, idbf=idbf, idf32=idf32, invcnt=invcnt, fcw=fcw,
            )
        )
    res = run_bass_kernel_spmd(
        nc, in_maps, list(range(NCORES)), trace=_trace
    )
    out = res.results[0]["logits"].astype(np.float32)
    if _want_results:
        return out, res
    return out


# revision 12
# speedup vs baseline: 2.5064x; 1.0931x over previous
"""GAT (3-layer, 4-head) + global mean pool + FC on 8 Trainium2 NeuronCores.

Strategy (v2)
-------------
Nodes sharded contiguously across 8 cores (2500 each, padded to 2560; `batch`
is sorted so this is graph-aligned data parallelism). Per layer:
  1. Dense phase: H^T = W^T X^T on the PE; attention logits a_src/a_dst per
     node; H^T transposed into a per-node row table [h(interleaved)|a_src]
     and AllGathered so every core holds the full 20480-row table in HBM.
  2. Agg phase, per 128-dst window: ONE dma_gather fetches h+a_src rows for
     the window's edges (sorted by dst, chunked 128/chunk). Host-precomputed
     one-hot scatter matrices O / O^T (static edge structure) stream in via
     HWDGE. a_dst per edge = O^T @ a_dst_window on the PE; q = exp(lrelu(
     a_s+a_d)) on ACT; q folded into gathered rows with a single broadcast
     multiply per chunk (head-interleaved channels); PE contracts
     out[dst,c] += O^T(edges->dst) . (q*h) plus denominators. Normalize +
     ELU fused on full-width [128,512] ops with per-partition reciprocals.
  3. Next layer's X^T obtained by HWDGE dma-transpose of the row output.
Final: graph mean-pool fused into layer-3 agg windows, AllReduce, FC.
"""
import os
import sys

sys.path.insert(0, "/opt/trn_rl_repo")

import ml_dtypes
import numpy as np

import concourse.bass as bass
import concourse.tile as tile
from concourse import bacc, mybir
from concourse.bass_utils import run_bass_kernel_spmd

# problem constants (hardcoded per the harness contract)
N = 20000
E0 = 320000
IN_CH = 256
HID = 128
HEADS = 4
OUT_CH = 200
G = 64
NEG_SLOPE = 0.2
NCORES = 8
SH = N // NCORES          # 2500 nodes per core
NW = (SH + 127) // 128    # 20 windows per core
SHP = NW * 128            # 2560 padded shard
NP = NCORES * SHP         # 20480 padded global rows
P = 128
ROW1 = 640                # table row cols (bf16) for layers 1-2: 512 h + 8 as-f32 + pad
ROW3 = 256                # layer 3: 128 h + 2 as-f32 + pad

F32 = mybir.dt.float32
BF16 = mybir.dt.bfloat16
I16 = mybir.dt.int16
BF = ml_dtypes.bfloat16

AluOp = mybir.AluOpType
Act = mybir.ActivationFunctionType


# ----------------------------------------------------------------- host prep
def preprocess(edge_index, batch):
    src = np.concatenate([edge_index[0].astype(np.int64), np.arange(N)])
    dst = np.concatenate([edge_index[1].astype(np.int64), np.arange(N)])
    order = np.argsort(dst, kind="stable")
    src_s = src[order]
    dst_s = dst[order]

    core = dst_s // SH
    win = (dst_s % SH) // 128
    group = core * NW + win                      # 0..159, nondecreasing
    counts = np.bincount(group, minlength=NCORES * NW)
    K = int(np.ceil(counts.max() / 128))
    KW = [
        max(1, int(np.ceil(counts.reshape(NCORES, NW)[:, w].max() / 128)))
        for w in range(NW)
    ]
    SLOTS = NW * K * 128

    starts = np.zeros(NCORES * NW, np.int64)
    starts[1:] = np.cumsum(counts)[:-1]
    rank = np.arange(len(dst_s)) - starts[group]
    slot = group * (K * 128) + rank              # global slot id

    SRC = np.zeros(NCORES * SLOTS, np.int64)
    DCOL = np.full(NCORES * SLOTS, -1, np.int64)
    # remap src node id into the padded 2560-per-core row space
    SRC[slot] = (src_s // SH) * SHP + (src_s % SH)
    DCOL[slot] = dst_s - core * SH - win * 128

    def wrap16(a):
        # slot i -> [i%16, i//16], replicated to 128 partitions
        w = a.reshape(-1, 16).T.astype(np.int16)     # [16, SLOTS/16]
        return np.ascontiguousarray(np.tile(w, (8, 1)))

    dkeys = np.arange(128, dtype=np.int64)
    per_core = []
    for c in range(NCORES):
        sl = slice(c * SLOTS, (c + 1) * SLOTS)
        srcidx = wrap16(SRC[sl])                      # [128, SLOTS/16] i16
        dcol = DCOL[sl].reshape(NW, K, 128)           # [w, k, p]
        # O[p, w, k, d] = 1 iff dst col of slot (w,k,p) == d
        oh = (dcol[:, :, :, None] == dkeys).astype(np.float32)  # [w,k,p,d]
        O = np.ascontiguousarray(oh.transpose(2, 0, 1, 3)).astype(BF)
        OT = np.ascontiguousarray(oh.transpose(3, 0, 1, 2)).astype(BF)
        nodes = c * SH + np.arange(SHP)
        gid = np.where(nodes < (c + 1) * SH, batch[np.minimum(nodes, N - 1)], -1)
        gidcol = gid.reshape(NW, 128).T.astype(np.float32)  # [128, NW]
        per_core.append(dict(srcidx=srcidx, O=O, OT=OT, gidcol=gidcol))
    cnts = np.bincount(batch.astype(np.int64), minlength=G).astype(np.float32)
    invcnt = (1.0 / np.maximum(cnts, 1.0)).reshape(G, 1)
    return K, KW, per_core, invcnt


# ------------------------------------------------------------ device program
def build_program(K, KW):
    nc = bacc.Bacc("TRN2", num_devices=NCORES)
    IDXW = NW * K * 128 // 16   # idx cols per core

    # ---- inputs
    xT0 = nc.dram_tensor("xT0", [P, 2, SHP], BF16, kind="ExternalInput")
    w1 = nc.dram_tensor("w1", [P, 2, 512], BF16, kind="ExternalInput")
    w2 = nc.dram_tensor("w2", [P, 4, 512], BF16, kind="ExternalInput")
    w3 = nc.dram_tensor("w3", [P, 4, 128], BF16, kind="ExternalInput")
    a1 = nc.dram_tensor("a1", [P, 8], BF16, kind="ExternalInput")
    a2 = nc.dram_tensor("a2", [P, 8], BF16, kind="ExternalInput")
    a3 = nc.dram_tensor("a3", [P, 2], BF16, kind="ExternalInput")
    srcidx_d = nc.dram_tensor("srcidx", [P, IDXW], I16, kind="ExternalInput")
    O_d = nc.dram_tensor("Omat", [P, NW, K, 128], BF16, kind="ExternalInput")
    OT_d = nc.dram_tensor("OTmat", [P, NW, K, 128], BF16, kind="ExternalInput")
    gidcol_d = nc.dram_tensor("gidcol", [P, NW], F32, kind="ExternalInput")
    iota64_d = nc.dram_tensor("iota64", [P, G], BF16, kind="ExternalInput")
    idbf_d = nc.dram_tensor("idbf", [P, P], BF16, kind="ExternalInput")
    idf32_d = nc.dram_tensor("idf32", [P, P], F32, kind="ExternalInput")
    invcnt_d = nc.dram_tensor("invcnt", [G, 1], F32, kind="ExternalInput")
    fcw_d = nc.dram_tensor("fcw", [HID, OUT_CH], F32, kind="ExternalInput")
    out_d = nc.dram_tensor("logits", [G, OUT_CH], F32, kind="ExternalOutput")
    hdump_d = nc.dram_tensor("hdump", [NP, ROW1], BF16, kind="ExternalOutput")
    xdump_d = nc.dram_tensor("xdump", [SHP, 512], BF16, kind="ExternalOutput")

    # ---- DRAM internals / collective buffers
    hag_in = [
        nc.dram_tensor(f"hag_in{l}", [SHP, ROW1 if l < 2 else ROW3], BF16)
        for l in range(3)
    ]
    hag_out = [
        nc.dram_tensor(
            f"hag_out{l}", [NP, ROW1 if l < 2 else ROW3], BF16,
            addr_space="Shared",
        )
        for l in range(3)
    ]
    xrows = [nc.dram_tensor(f"xrows{l}", [SHP, 512], BF16) for l in (1, 2)]
    ar_in = nc.dram_tensor("ar_in", [G, HID], F32)
    ar_out = nc.dram_tensor("ar_out", [G, HID], F32, addr_space="Shared")

    RG = [list(range(NCORES))]
    NT = SHP // 512  # 5 tiles per shard in the dense phase
    dma_sem = nc.alloc_semaphore("swdge_dma")

    with tile.TileContext(nc) as tc:
        with (
            tc.tile_pool(name="const", bufs=1) as cpool,
            tc.tile_pool(name="xbuf", bufs=1) as xpool,
            tc.tile_pool(name="dense", bufs=2) as dpool,
            tc.tile_pool(name="gather", bufs=2) as gpool,
            tc.tile_pool(name="work", bufs=2) as wpool,
            tc.tile_pool(name="psA", bufs=1, space="PSUM") as psA,
        ):
            # ---- constants into SBUF
            def load_const(dram, shape, dt, name):
                t = cpool.tile(shape, dt, name=name)
                nc.sync.dma_start(t[:], dram[:])
                return t

            iota64 = load_const(iota64_d, [P, G], BF16, "iota64")
            idbf = load_const(idbf_d, [P, P], BF16, "idbf")
            idf32 = load_const(idf32_d, [P, P], F32, "idf32")
            invcnt = load_const(invcnt_d, [G, 1], F32, "invcnt")
            fcw = load_const(fcw_d, [HID, OUT_CH], F32, "fcw")
            a_sb = [
                load_const(d, [P, n], BF16, f"a{i+1}")
                for i, (d, n) in enumerate([(a1, 8), (a2, 8), (a3, 2)])
            ]
            w_sb = [
                load_const(w1, [P, 2, 512], BF16, "w1"),
                load_const(w2, [P, 4, 512], BF16, "w2"),
                load_const(w3, [P, 4, 128], BF16, "w3"),
            ]
            srcidx = load_const(srcidx_d, [P, IDXW], I16, "srcidx")
            gidcol = load_const(gidcol_d, [P, NW], F32, "gidcol")
            zero1 = cpool.tile([P, 1], F32, name="zero1")
            nc.vector.memset(zero1[:], 0.0)

            # persistent SBUF buffers
            xT = xpool.tile([P, 4, SHP], BF16, name="xT")
            nc.sync.dma_start(xT[:, 0:2, :], xT0[:])
            ht = xpool.tile([P, 4, SHP], BF16, name="ht")
            adrec = xpool.tile([P, NW, HEADS], BF16, name="adrec")

            def dense_phase(l):
                """X^T -> H^T; alpha logits; row table -> hag_in; AllGather."""
                cinb = [2, 4, 4][l]
                coutb = [4, 4, 1][l]
                nh = [4, 4, 1][l]
                rowc = ROW1 if l < 2 else ROW3
                # H^T = W^T @ X^T
                for co in range(coutb):
                    for t in range(NT):
                        pm = psA.tile([P, 512], F32, tag=f"a{t % 2}",
                                      name=f"mm{l}_{co}_{t}")
                        for k in range(cinb):
                            nc.tensor.matmul(
                                out=pm[:],
                                lhsT=w_sb[l][:, k, co * 128: co * 128 + 128],
                                rhs=xT[:, k, t * 512: (t + 1) * 512],
                                start=(k == 0),
                                stop=(k == cinb - 1),
                            )
                        nc.vector.tensor_copy(
                            ht[:, co, t * 512: (t + 1) * 512], pm[:]
                        )
                # alpha logits [2, SHP] f32 per head
                ast_h = []
                for h in range(nh):
                    ah = dpool.tile([2, SHP], BF16, tag=f"ast{h}", bufs=1,
                                    name=f"ast{l}_{h}")
                    for t in range(NT):
                        pa = psA.tile([2, 512], F32, tag=f"c{t % 2}",
                                      name=f"aps{l}_{h}_{t}")
                        nc.tensor.matmul(
                            out=pa[:],
                            lhsT=a_sb[l][:, 2 * h: 2 * h + 2],
                            rhs=ht[:, h, t * 512: (t + 1) * 512],
                            start=True,
                            stop=True,
                        )
                        nc.vector.tensor_copy(ah[:, t * 512: (t + 1) * 512], pa[:])
                    ast_h.append(ah)
                # per-window: transpose H^T into interleaved rows + a-records
                for w in range(NW):
                    ws = slice(w * 128, (w + 1) * 128)
                    rows = dpool.tile([P, rowc], BF16, tag="rows",
                                      name=f"rows{l}_{w}")
                    rview = (
                        rows[:, 0:512].rearrange("p (c h) -> p c h", h=4)
                        if nh == 4 else rows[:, 0:128]
                    )
                    for co in range(coutb):
                        pt = psA.tile([P, P], BF16, tag=f"b{co % 2}",
                                      name=f"htp{l}_{w}_{co}")
                        nc.tensor.matmul(
                            out=pt[:], lhsT=ht[:, co, ws], rhs=idbf[:],
                            start=True, stop=True, is_transpose=True,
                        )
                        if nh == 4:
                            nc.vector.tensor_copy(rview[:, :, co], pt[:])
                        else:
                            nc.vector.tensor_copy(rview[:, :], pt[:])
                    # a_src / a_dst records: transpose [2,128] -> [128,2] bf16
                    for h in range(nh):
                        pr = psA.tile([P, 2], BF16, tag=f"c{h % 2}",
                                      name=f"arec{l}_{w}_{h}")
                        nc.tensor.matmul(
                            out=pr[:], lhsT=ast_h[h][:, ws], rhs=idbf[:2, :2],
                            start=True, stop=True, is_transpose=True,
                        )
                        nc.vector.tensor_copy(
                            rows[:, 512 + h: 513 + h] if l < 2
                            else rows[:, 128:129],
                            pr[:, 0:1],
                        )
                        nc.vector.tensor_copy(adrec[:, w, h: h + 1], pr[:, 1:2])
                    nc.sync.dma_start(hag_in[l][ws, :], rows[:])
                nc.gpsimd.collective_compute(
                    "AllGather", AluOp.bypass, replica_groups=RG,
                    ins=[hag_in[l][:]], outs=[hag_out[l][:]],
                )

            def agg_phase(l, pool_ps=None):
                """Gather + attention + scatter; rows out (elu'd)."""
                nh = [4, 4, 1][l]
                C = [512, 512, 128][l]
                rowc = ROW1 if l < 2 else ROW3
                for w in range(NW):
                    Kw = KW[w]
                    NI = Kw * 128
                    isl = slice(w * K * 8, w * K * 8 + Kw * 8)
                    hg = gpool.tile([P, Kw, rowc], BF16, tag="hg", name=f"hg{l}_{w}")
                    nc.gpsimd.dma_gather(
                        hg[:], hag_out[l][:], srcidx[:, isl], NI, NI, rowc,
                        single_packet=False,
                    )
                    Ow = gpool.tile([P, Kw, 128], BF16, tag="Ow", name=f"O{l}_{w}")
                    nc.sync.dma_start(Ow[:], O_d[:, w, 0:Kw, :])
                    OTw = gpool.tile([P, Kw, 128], BF16, tag="OTw", name=f"OT{l}_{w}")
                    nc.sync.dma_start(OTw[:], OT_d[:, w, 0:Kw, :])
                    # a_dst per edge via O^T @ ad_window  -> [128, K, nh] psum
                    adps = psA.tile([P, Kw * nh], F32, tag=f"b{w % 2}",
                                    name=f"adps{l}_{w}")
                    for k in range(Kw):
                        nc.tensor.matmul(
                            out=adps[:, k * nh: (k + 1) * nh],
                            lhsT=OTw[:, k, :], rhs=adrec[:, w, 0:nh],
                            start=True, stop=True,
                        )
                    # q = exp(lrelu(as + ad)) -> bf16 [128, K, nh]
                    asv = (
                        hg[:, :, 512:516] if l < 2 else hg[:, :, 128:129]
                    )  # [128, K, nh] bf16
                    tq = wpool.tile([P, Kw, nh], F32, tag="tq", name=f"tq{l}_{w}")
                    nc.vector.tensor_tensor(
                        out=tq[:], in0=asv,
                        in1=adps[:].rearrange("p (k h) -> p k h", h=nh),
                        op=AluOp.add,
                    )
                    ql = wpool.tile([P, Kw, nh], F32, tag="ql", name=f"ql{l}_{w}")
                    nc.vector.scalar_tensor_tensor(
                        out=ql[:], in0=tq[:], scalar=NEG_SLOPE, in1=tq[:],
                        op0=AluOp.mult, op1=AluOp.max,
                    )
                    qf = wpool.tile([P, Kw, nh], BF16, tag="qf", name=f"qf{l}_{w}")
                    nc.scalar.activation(qf[:], ql[:], Act.Exp)
                    if nh == 1:
                        qf32 = wpool.tile([P, Kw, 1], F32, tag="qf32",
                                          name=f"qf32{l}_{w}")
                        nc.scalar.activation(qf32[:], ql[:], Act.Exp)
                    # hgs = hg * q (broadcast over channels), per chunk
                    hgs = wpool.tile([P, Kw, C], BF16, tag="hgs", bufs=1, name=f"hgs{l}_{w}")
                    pagg = psA.tile([P, C], F32, tag=f"a{w % 2}", name=f"pagg{l}_{w}")
                    den = psA.tile([P, nh], F32, tag=f"c{w % 2}", name=f"den{l}_{w}")
                    for k in range(Kw):
                        if nh == 4:
                            nc.vector.tensor_tensor(
                                out=hgs[:, k, :].rearrange("p (c h) -> p c h", h=4),
                                in0=hg[:, k, 0:512].rearrange("p (c h) -> p c h", h=4),
                                in1=qf[:, k, :].unsqueeze(1).broadcast_to(
                                    [P, 128, 4]
                                ),
                                op=AluOp.mult,
                            )
                        else:
                            nc.vector.tensor_tensor(
                                out=hgs[:, k, :], in0=hg[:, k, 0:128],
                                in1=qf32[:, k, 0:1].broadcast_to([P, 128]),
                                op=AluOp.mult,
                            )
                        nc.tensor.matmul(
                            out=pagg[:], lhsT=Ow[:, k, :], rhs=hgs[:, k, :],
                            start=(k == 0), stop=(k == Kw - 1),
                        )
                        nc.tensor.matmul(
                            out=den[:], lhsT=Ow[:, k, :], rhs=qf[:, k, :],
                            start=(k == 0), stop=(k == Kw - 1),
                        )
                    # normalize + elu -> rows (bf16)
                    rec = wpool.tile([P, nh], F32, tag="rec", name=f"rec{l}_{w}")
                    nc.vector.scalar_tensor_tensor(
                        out=rec[:], in0=den[:], scalar=1e-16, in1=zero1[:, 0:1].broadcast_to([P, nh]),
                        op0=AluOp.add, op1=AluOp.add,
                    )
                    nc.vector.reciprocal(rec[:], rec[:])
                    tmul = wpool.tile([P, C], F32, tag="tmul", bufs=1, name=f"tm{l}_{w}")
                    if nh == 4:
                        nc.vector.tensor_tensor(
                            out=tmul[:].rearrange("p (c h) -> p c h", h=4),
                            in0=pagg[:].rearrange("p (c h) -> p c h", h=4),
                            in1=rec[:].unsqueeze(1).broadcast_to([P, 128, 4]),
                            op=AluOp.mult,
                        )
                    else:
                        nc.vector.tensor_tensor(
                            out=tmul[:], in0=pagg[:],
                            in1=rec[:, 0:1].broadcast_to([P, 128]),
                            op=AluOp.mult,
                        )
                    tmin = wpool.tile([P, C], F32, tag="tmin", bufs=1, name=f"tn{l}_{w}")
                    nc.vector.scalar_tensor_tensor(
                        out=tmin[:], in0=tmul[:], scalar=0.0,
                        in1=zero1[:, 0:1].broadcast_to([P, C]),
                        op0=AluOp.add, op1=AluOp.min,
                    )
                    em = wpool.tile([P, C], F32, tag="em", bufs=1, name=f"em{l}_{w}")
                    nc.scalar.activation(em[:], tmin[:], Act.Exp)
                    relu = wpool.tile([P, C], F32, tag="relu", bufs=1, name=f"rl{l}_{w}")
                    nc.vector.scalar_tensor_tensor(
                        out=relu[:], in0=tmul[:], scalar=0.0,
                        in1=zero1[:, 0:1].broadcast_to([P, C]),
                        op0=AluOp.add, op1=AluOp.max,
                    )
                    orow = wpool.tile([P, C], BF16, tag="orow", name=f"or{l}_{w}")
                    nc.vector.scalar_tensor_tensor(
                        out=orow[:], in0=em[:], scalar=-1.0, in1=relu[:],
                        op0=AluOp.add, op1=AluOp.add,
                    )
                    if l < 2:
                        nc.sync.dma_start(xrows[l][w * 128:(w + 1) * 128, :], orow[:])
                    else:
                        # fuse graph pooling: pool_ps += gsel^T @ rows
                        gw = wpool.tile([P, G], BF16, tag="gw", name=f"gw_{w}")
                        nc.vector.tensor_tensor(
                            out=gw[:], in0=iota64[:],
                            in1=gidcol[:, w: w + 1].broadcast_to([P, G]),
                            op=AluOp.is_equal,
                        )
                        nc.tensor.matmul(
                            out=pool_ps[:], lhsT=gw[:], rhs=orow[:],
                            start=(w == 0), stop=(w == NW - 1),
                        )

            def load_xT(l):
                """X^T for layer l in {1,2} via HWDGE dma-transpose of rows."""
                for b in range(4):
                    nc.sync.dma_start_transpose(
                        xT[:, b, :], xrows[l - 1][:, b * 128:(b + 1) * 128]
                    )

            def pool_fc(pool_ps):
                psums = wpool.tile([G, HID], F32, tag="psums", name="psums")
                nc.vector.tensor_copy(psums[:], pool_ps[:])
                nc.sync.dma_start(ar_in[:], psums[:])
                nc.gpsimd.collective_compute(
                    "AllReduce", AluOp.add, replica_groups=RG,
                    ins=[ar_in[:]], outs=[ar_out[:]],
                )
                sums = wpool.tile([G, HID], F32, tag="sums", name="sums")
                nc.sync.dma_start(sums[:], ar_out[:])
                pooled = wpool.tile([G, HID], F32, tag="pooled", name="pooled")
                nc.vector.tensor_scalar(
                    out=pooled[:], in0=sums[:], scalar1=invcnt[:, 0:1],
                    scalar2=None, op0=AluOp.mult,
                )
                ptp = psA.tile([HID, G], F32, tag="c0", name="poolT")
                nc.tensor.matmul(
                    out=ptp[:], lhsT=pooled[:], rhs=idf32[:G, :G],
                    start=True, stop=True, is_transpose=True,
                )
                poolT = wpool.tile([HID, G], F32, tag="poolT", name="poolTs")
                nc.vector.tensor_copy(poolT[:], ptp[:])
                pfc = psA.tile([G, OUT_CH], F32, tag="b0", name="fcps")
                nc.tensor.matmul(
                    out=pfc[:], lhsT=poolT[:], rhs=fcw[:], start=True, stop=True
                )
                logits = wpool.tile([G, OUT_CH], F32, tag="logits", name="logits")
                nc.vector.tensor_copy(logits[:], pfc[:])
                nc.sync.dma_start(out_d[:], logits[:])

            dense_phase(0)
            agg_phase(0)
            load_xT(1)
            dense_phase(1)
            agg_phase(1)
            load_xT(2)
            dense_phase(2)
            pool_ps = psA.tile([G, HID], F32, tag="d0", name="poolps")
            agg_phase(2, pool_ps)
            pool_fc(pool_ps)

            if os.environ.get("DUMP_H"):
                li = int(os.environ["DUMP_H"])
                cw = ROW1 if li < 2 else ROW3
                hstg = wpool.tile([P, cw], BF16, tag="hdmp", bufs=2, name="hdmp")
                for b in range(NP // P):
                    lo, hi = b * P, (b + 1) * P
                    nc.sync.dma_start(hstg[:], hag_out[li][lo:hi, :])
                    nc.sync.dma_start(hdump_d[lo:hi, 0:cw], hstg[:])
            if os.environ.get("DUMP_X"):
                xi = int(os.environ["DUMP_X"])  # 1 or 2: xrows after agg xi-1
                xstg = wpool.tile([P, 512], BF16, tag="xdmp", bufs=2, name="xdmp")
                for b in range(SHP // P):
                    lo, hi = b * P, (b + 1) * P
                    nc.sync.dma_start(xstg[:], xrows[xi - 1][lo:hi, :])
                    nc.sync.dma_start(xdump_d[lo:hi, :], xstg[:])

    nc.compile()
    return nc


_prog_cache = {}


def _interleave_perm():
    # perm[j] = flat channel index stored at interleaved col j
    j = np.arange(512)
    c, h = j // 4, j % 4
    return h * 128 + c


def kernel(x, edge_index, batch, W1, a_src1, a_dst1, b1,
           W2, a_src2, a_dst2, b2, W3, a_src3, a_dst3, b3, fc_w, fc_b,
           _want_results=False, _trace=False):
    x = np.asarray(x)
    edge_index = np.asarray(edge_index)
    batch = np.asarray(batch)
    for b in (b1, b2, b3, fc_b):
        assert not np.any(np.asarray(b)), "nonzero biases not supported"

    K, KW, per_core, invcnt = preprocess(edge_index, batch)
    ck = (K, tuple(KW))
    if ck not in _prog_cache:
        _prog_cache[ck] = build_program(K, KW)
    nc = _prog_cache[ck]

    iota64 = np.ascontiguousarray(
        np.broadcast_to(np.arange(G, dtype=np.float32), (P, G)).astype(BF)
    )
    idbf = np.eye(P, dtype=np.float32).astype(BF)
    idf32 = np.eye(P, dtype=np.float32)
    perm = _interleave_perm()

    def wmat(W, cinb, cout, perm_in=None):
        Wf = np.asarray(W, np.float32)
        if perm_in is not None:
            Wf = Wf[perm_in]
        return np.ascontiguousarray(
            Wf.reshape(cinb, 128, cout).transpose(1, 0, 2)
        ).astype(BF)

    w1m = wmat(W1, 2, 512)
    w2m = wmat(W2, 4, 512, perm)
    w3m = wmat(W3, 4, 128, perm)

    def avec(asrc, adst):
        nh = asrc.shape[0]
        out = np.empty((128, 2 * nh), np.float32)
        out[:, 0::2] = np.asarray(asrc, np.float32).T
        out[:, 1::2] = np.asarray(adst, np.float32).T
        return np.ascontiguousarray(out).astype(BF)

    a1m = avec(a_src1, a_dst1)
    a2m = avec(a_src2, a_dst2)
    a3m = avec(a_src3, a_dst3)
    fcw = np.ascontiguousarray(np.asarray(fc_w, np.float32))

    xf = np.asarray(x, np.float32)
    in_maps = []
    for c in range(NCORES):
        xs = np.zeros((IN_CH, SHP), np.float32)
        xs[:, :SH] = xf[c * SH: (c + 1) * SH].T
        pc = per_core[c]
        in_maps.append(
            dict(
                xT0=np.ascontiguousarray(
                    xs.reshape(2, 128, SHP).transpose(1, 0, 2)
                ).astype(BF),
                w1=w1m, w2=w2m, w3=w3m, a1=a1m, a2=a2m, a3=a3m,
                srcidx=pc["srcidx"], Omat=pc["O"], OTmat=pc["OT"],
                gidcol=pc["gidcol"],
                iota64=iota64, idbf=idbf, idf32=idf32, invcnt=invcnt, fcw=fcw,
            )
        )
    res = run_bass_kernel_spmd(
        nc, in_maps, list(range(NCORES)), trace=_trace
    )
    out = res.results[0]["logits"].astype(np.float32)
    if _want_results:
        return out, res
    return out
